# revision 1
# baseline (speedup 1.0000x reference)
"""Trainium2 Bass kernel for nn_ASCGM_30090540876360 (3x3 median-trimmed residual
between two 1x1 convs).

Math: reference computes, per (b,c,h,w), over the 9-point reflect-padded
neighborhood of d = conv1x1(x):
    diff_k = n_k - c ; absd_k = |diff_k| ; med = median9(absd)
    keep absd<=med, s = absd/max(kept absd); d3 = sum(diff*(1-s))
Since the center diff is always 0, med = 4th-smallest of the 8 neighbor
|diffs|, max(kept absd) = med, and elements with absd == med contribute 0.
Therefore exactly:
    d3 = T1 - R/med,  T1 = sum_k diff_k,  R = sum_k diff_k * min(absd_k, med)
A tiny floor on the median (med = max(med, 1e-4)) keeps the formula finite
and correct when fp16 rounding creates >=4 zero diffs (reflected corners):
there d3 degrades gracefully to T1 - sum(nonzero diffs) = 0, matching the
reference's behavior at such near-degenerate pixels.

Sharding: data-parallel over batch B=8 across the 8 NeuronCores (1 image per
core).  On-core layout: 128 partitions = 2 image halves x 64 channels; each
partition holds 64 rows (+1 halo row each side) of one half.  Both halves are
processed by single instructions via block-diagonal conv weights.

Stencil runs in fp16 (DVE 2x mode) against a dual-copy padded d buffer
(dpadE / dpadO shifted by one element) so every strided fp16 operand stays
4-byte aligned.
"""
import sys, os
sys.path.insert(0, '/opt/trn_rl_repo')

import numpy as np
from contextlib import ExitStack

import concourse.bass as bass
import concourse.tile as tile
from concourse import bacc, mybir
from concourse.bass_utils import run_bass_kernel_spmd
from concourse import dve_ops as _dve_ops
from concourse.dve_spec import Spec, Src0, Src1, Zero, maxx, minn, lower
from concourse.dve_spec import _has_src1 as has_src1
from concourse.dve_uop import DveOpSpec


def _register_medprod():
    """Custom DVE op: out = in0 * min(|in0|, in1)  (one pass, replaces 2)."""
    name = "ANT_MEDPROD_K"
    for op in _dve_ops.OPS:
        if op.name == name:
            return op
    import numpy as _np
    body = Src0 * minn(maxx(Src0, Zero - Src0), Src1)
    spec = Spec(body=body,
                reference=lambda in0, in1, *a: in0 * _np.minimum(
                    _np.abs(in0), in1.reshape(in0.shape) if in1.size == in0.size
                    else in1))
    shas = {}
    op = _dve_ops.DveOp(name, spec, subdim=False, uops_sha=shas)
    _dve_ops.OPS.append(op)
    _dve_ops._SUB_OPCODE_FOR_NAME[name] = (_dve_ops._CUSTOM_DVE_ROW_BASE
                                           + len(_dve_ops.OPS) - 1)
    _dve_ops.CUSTOM_DVE_SPECS[name] = spec
    for ver in ("v3", "v4"):
        r = DveOpSpec(name=name, opcode=_dve_ops.get_dve_sub_opcode(name),
                      uops=lower(spec, ver=ver), rd1_en=has_src1(spec))
        shas[ver] = r.sha(ver)
    return op


MEDPROD = _register_medprod()

F16 = mybir.dt.float16
F32 = mybir.dt.float32
ALU = mybir.AluOpType
AFT = mybir.ActivationFunctionType

C = 64          # channels
H = W = 128     # image size
NCORES = 8
PR = 66         # padded rows per half (64 + halo)
WP = 130        # padded row width
ETA = 1e-4      # median floor (fp16-safe; see module docstring)
RCH = 16        # stencil chunk rows (per half) -> 4 chunks
NB8 = [(0, 0), (0, 2), (2, 0), (2, 2), (1, 0), (1, 2), (0, 1), (2, 1)]


def build_program():
    nc = bacc.Bacc("TRN2", target_bir_lowering=False, debug=False)

    x16 = nc.dram_tensor("x16", [C, H, W], F16, kind="ExternalInput")
    w1bd = nc.dram_tensor("w1bd", [128, 128], F16, kind="ExternalInput")
    w2abd = nc.dram_tensor("w2abd", [128, 128], F16, kind="ExternalInput")
    w2bbd = nc.dram_tensor("w2bbd", [128, 128], F16, kind="ExternalInput")
    b1v = nc.dram_tensor("b1v", [128, 1], F32, kind="ExternalInput")
    b2v = nc.dram_tensor("b2v", [128, 1], F32, kind="ExternalInput")
    out = nc.dram_tensor("out", [C, H, W], F32, kind="ExternalOutput")

    v = nc.vector
    s = nc.scalar

    with tile.TileContext(nc) as tc:
        with ExitStack() as ctx:
            cpool = ctx.enter_context(tc.tile_pool(name="const", bufs=1))
            w1sb = cpool.tile([128, 128], F16, tag="w1sb")
            w2asb = cpool.tile([128, 128], F16, tag="w2asb")
            w2bsb = cpool.tile([128, 128], F16, tag="w2bsb")
            b1sb = cpool.tile([128, 1], F32, tag="b1sb")
            b2sb = cpool.tile([128, 1], F32, tag="b2sb")

            dpool = ctx.enter_context(tc.tile_pool(name="dpad", bufs=1))
            dpadE = dpool.tile([128, PR, WP], F16, tag="dpadE")
            dpadO = dpool.tile([128, PR, WP], F16, tag="dpadO")
            dvpad = dpool.tile([128, 64, WP], F16, tag="dvpad")
            b1x3 = cpool.tile([128, 1], F32, name="b1x3", tag="b1x3")

            # ---- load x with halo rows (reflection handled by duplicate DMAs)
            xpool = ctx.enter_context(tc.tile_pool(name="xp", bufs=1))
            xsb = xpool.tile([128, PR, W], F16, tag="xsb")
            # half A: global rows -1..64 -> local 0..65 (row -1 == row 1)
            # half B: global rows 63..128 -> local 0..65 (row 128 == row 126)
            # First pieces small (conv chunk 0 needs only local rows 0..5);
            # the bulk rides the separate SWDGE queue so issues overlap.
            # issue order = critical path: the x rows and w1 gate the first
            # matmul (w1's transfer is tiny; b1 is only needed by the first
            # evacuation, slightly later); everything else after
            nc.sync.dma_start(xsb[0:64, 1:20, :], x16[:, 0:19, :])
            nc.sync.dma_start(xsb[64:128, 0:20, :], x16[:, 63:83, :])
            nc.sync.dma_start(xsb[0:64, 0:1, :], x16[:, 1:2, :])
            nc.sync.dma_start(w1sb[:], w1bd[:])
            nc.sync.dma_start(b1sb[:], b1v[:])
            s.mul(b1x3[:], b1sb[:], 3.0)   # on ACT: keeps DVE free at start
            nc.sync.dma_start(w2asb[:], w2abd[:])
            nc.sync.dma_start(w2bsb[:], w2bbd[:])
            nc.sync.dma_start(b2sb[:], b2v[:])
            nc.sync.dma_start(xsb[64:128, 65:66, :], x16[:, 126:127, :])
            for rr in range(19, 65, 16):  # bulk loads, alternating queues
                ra = min(rr + 16, 65)   # half A: local 1+rr <- global rr
                rb = min(rr + 16, 64)   # half B: local 1+rr <- global 64+rr
                nc.gpsimd.dma_start(xsb[0:64, 1 + rr:1 + ra, :],
                                    x16[:, rr:ra, :])
                if rb > rr:
                    nc.sync.dma_start(xsb[64:128, 1 + rr:1 + rb, :],
                                      x16[:, 64 + rr:64 + rb, :])

            # ---- conv1 producers (emitted per consumer chunk, see loop)
            pp1 = ctx.enter_context(tc.tile_pool(name="psum1", bufs=2,
                                                 space="PSUM"))
            xflat = xsb[:].rearrange("p r w -> p (r w)")
            NTOT = PR * W  # 8448

            def conv1_chunk(cc, e_on_dve=False):
                # conv1 of padded rows 4cc..4cc+3 (last chunk: 2 rows)
                n0 = 512 * cc
                nsz = min(512, NTOT - n0)
                nr = nsz // W
                r0 = 4 * cc
                ps = pp1.tile([128, nr, W], F32, name="ps1", tag="ps1")
                nc.tensor.matmul(ps[:], w1sb[:], xflat[:, n0:n0 + nsz],
                                 start=True, stop=True)
                # evacuate with bias, fp32->fp16, into both shifted pads,
                # including the reflected column pads straight from PSUM
                s.add(dpadE[:, r0:r0 + nr, 1:129], ps[:], b1sb[:])
                if e_on_dve:
                    # prologue only: DVE is idle, so build the odd-shifted
                    # copy from dpadE there and shorten ACT's critical path;
                    # column pads for these rows are emitted afterwards from
                    # dpadE (not PSUM) so they neither occupy ACT's queue
                    # between the gating E-evacuations nor hold PSUM slots
                    v.tensor_copy(dpadO[:, r0:r0 + nr, 2:130],
                                  dpadE[:, r0:r0 + nr, 1:129])
                else:
                    s.add(dpadO[:, r0:r0 + nr, 2:130], ps[:], b1sb[:])
                    s.add(dpadE[:, r0:r0 + nr, 0:130:129],
                          ps[:, :, 1:127:125], b1sb[:])

            def dv_chunk(ch):
                # dv = vertical-3-sum of d (PSUM accumulation over row-shifted
                # rhs views); interior rows 4ch..4ch+3
                m0 = W + 512 * ch
                psv = pp1.tile([128, 4, W], F32, name="psv", tag="psv")
                nc.tensor.matmul(psv[:], w1sb[:],
                                 xflat[:, m0 - W:m0 - W + 512],
                                 start=True, stop=False)
                nc.tensor.matmul(psv[:], w1sb[:], xflat[:, m0:m0 + 512],
                                 start=False, stop=False)
                nc.tensor.matmul(psv[:], w1sb[:],
                                 xflat[:, m0 + W:m0 + W + 512],
                                 start=False, stop=True)
                s.add(dvpad[:, 4 * ch:4 * ch + 4, 1:129], psv[:], b1x3[:])
                s.add(dvpad[:, 4 * ch:4 * ch + 4, 0:130:129],
                      psv[:, :, 1:127:125], b1x3[:])

            # conv1 chunk ranges produced right before the stencil chunk that
            # first needs them (software pipelining via emission order)
            CONV_RANGES = [(0, 5), (5, 9), (9, 13), (13, 17)]
            DV_RANGES = [(0, 4), (4, 8), (8, 12), (12, 16)]

            # ---- stencil + conv2, chunked over rows
            spool = ctx.enter_context(tc.tile_pool(name="sten", bufs=1))
            opool = ctx.enter_context(tc.tile_pool(name="outp", bufs=1))
            pp2 = ctx.enter_context(tc.tile_pool(name="psum2", bufs=4, space="PSUM"))

            def nb_view(i, j, r0, nr):
                # neighborhood view (i,j) for chunk local padded rows r0..r0+nr
                if j == 1:
                    return dpadO[:, r0 + i:r0 + i + nr, 2:130]
                return dpadE[:, r0 + i:r0 + i + nr, j:j + 128]

            for ci in range(64 // RCH):
                # conv chunks first: they gate the stencil diffs; dv only
                # feeds T1 which is consumed late in the chunk
                for cc in range(*CONV_RANGES[ci]):
                    conv1_chunk(cc, e_on_dve=(ci == 0))
                if ci == 0:
                    # deferred prologue column pads, sourced from dpadE
                    c0, c1_ = CONV_RANGES[0]
                    s.copy(dpadE[:, 4 * c0:4 * c1_, 0:130:129],
                           dpadE[:, 4 * c0:4 * c1_, 2:128:125])
                for ch in range(*DV_RANGES[ci]):
                    dv_chunk(ch)

                rr0 = 1 + ci * RCH          # first interior padded row of chunk
                ctr = dpadO[:, rr0:rr0 + RCH, 2:130]

                def newt(tag, dt=F16):
                    return spool.tile([128, RCH, W], dt, name=tag, tag=tag)

                dstack = spool.tile([128, 8, RCH, W], F16, name="dstack",
                                    tag="dstack")
                diff = [dstack[:, k] for k in range(8)]
                # 4 fused subtracts, 2 neighbors each (k-dim in the AP); the
                # center operand broadcasts over k with a 0-stride dim.
                # For the first chunk, emit in two row-halves so the first
                # half's diffs start as soon as the first conv chunks land.
                eflat = dpadE[:].rearrange("p r w -> p (r w)")
                oflat = dpadO[:].rearrange("p r w -> p (r w)")
                pairs = [  # (src flat view, base row offset, k step)
                    (eflat, rr0 - 1, 0, 2),        # (0,0),(0,2)
                    (eflat, rr0 + 1, 0, 2),        # (2,0),(2,2)
                    (eflat, rr0, 0, 2),            # (1,0),(1,2)
                    (oflat, rr0 - 1, 2, 2 * WP),   # (0,1),(2,1)
                ]
                halves = [(0, RCH // 2), (RCH // 2, RCH - RCH // 2)] \
                    if ci == 0 else [(0, RCH)]
                for hr0, hnr in halves:
                    ctrb = bass.AP(tensor=oflat.tensor,
                                   offset=(rr0 + hr0) * WP + 2,
                                   ap=[oflat.ap[0], [0, 2], [WP, hnr], [1, W]])
                    for pi, (src, brow, bcol, kst) in enumerate(pairs):
                        nb2 = bass.AP(tensor=src.tensor,
                                      offset=(brow + hr0) * WP + bcol,
                                      ap=[src.ap[0], [kst, 2], [WP, hnr],
                                          [1, W]])
                        v.tensor_tensor(dstack[:, 2 * pi:2 * pi + 2,
                                               hr0:hr0 + hnr], nb2, ctrb,
                                        ALU.subtract)


                absd = []
                for k in range(8):
                    ak = newt(f"absd{k}")
                    s.activation(ak[:], diff[k][:], AFT.Abs)   # ACT (DVE offload)
                    absd.append(ak)

                def tt(op, a, b, o):
                    v.tensor_tensor(o[:], a[:], b[:], op)
                    return o

                # 25-op selection of 4th-smallest-of-8 via median-of-9
                # (the 9th value is the always-zero center diff):
                # med9 = med3( max3(mins), med3(mids), min3(maxs) ) over three
                # sorted triples T0=(0,a0,a1), T1=(a2,a3,a4), T2=(a5,a6,a7).
                # absd slots are reused as scratch once their value is dead.
                A = absd
                p0, q0 = newt("p0"), newt("q0")
                tt(ALU.min, A[0], A[1], p0); tt(ALU.max, A[0], A[1], q0)
                # sort3 of (A2,A3,A4) -> lo1=A4, mi1=A0, hi1=A2
                tt(ALU.min, A[2], A[3], A[0]); tt(ALU.max, A[2], A[3], A[1])
                tt(ALU.max, A[1], A[4], A[2]); tt(ALU.min, A[1], A[4], A[3])
                tt(ALU.min, A[0], A[3], A[4]); tt(ALU.max, A[0], A[3], A[0])
                # sort3 of (A5,A6,A7) -> lo2=A7, mi2=A1, hi2=A5
                tt(ALU.min, A[5], A[6], A[1]); tt(ALU.max, A[5], A[6], A[3])
                tt(ALU.max, A[3], A[7], A[5]); tt(ALU.min, A[3], A[7], A[6])
                tt(ALU.min, A[1], A[6], A[7]); tt(ALU.max, A[1], A[6], A[1])
                # combine
                tt(ALU.max, A[4], A[7], A[3])          # mxlo
                tt(ALU.min, A[2], A[5], A[6])          # min(hi1,hi2)
                tt(ALU.min, A[6], q0, A[6])            # mnhi
                tt(ALU.min, p0, A[0], A[4])            # m_ab
                tt(ALU.max, p0, A[0], A[2])            # M_ab
                tt(ALU.min, A[2], A[1], A[7])          # m_c
                tt(ALU.max, A[4], A[7], A[4])          # mdmid
                tt(ALU.min, A[3], A[4], A[0])          # f_ab
                tt(ALU.max, A[3], A[4], A[1])          # F_ab
                tt(ALU.min, A[1], A[6], A[2])          # f_c
                tt(ALU.max, A[0], A[2], A[5])          # med (pre-floor) in A5
                v3 = A[5]
                med = newt("med")
                v.tensor_scalar_max(med[:], v3[:], ETA)  # floor (v3 >= 0)

                # T1 = box9 - 9*center; vertical 3-sum came from PE (dvpad).
                # Emitted after the network: dv lands during the prologue's
                # first network stage without gating the diffs.
                cr0 = ci * RCH
                T1 = newt("T1")
                v.tensor_add(T1[:], dvpad[:, cr0:cr0 + RCH, 0:128],
                             dvpad[:, cr0:cr0 + RCH, 2:130])
                v.tensor_add(T1[:], T1[:], dvpad[:, cr0:cr0 + RCH, 1:129])
                n9c = newt("n9c")
                v.tensor_scalar_mul(n9c[:], ctr, -9.0)
                v.tensor_add(T1[:], T1[:], n9c[:])

                medf = newt("medf", F32)
                s.copy(medf[:], med[:])                    # ACT cast f16->f32
                v.reciprocal_approx_fast(medf[:], medf[:])  # in-place 1/med
                rmed = newt("rmed")
                s.copy(rmed[:], medf[:])                   # ACT cast f32->f16

                # R = sum_k diff_k * min(|diff_k|, med)  (fused custom op)
                mflat = med[:].rearrange("p r w -> p (r w)")
                medb = bass.AP(tensor=mflat.tensor, offset=mflat.offset,
                               ap=[mflat.ap[0], [0, 8], mflat.ap[1]])
                dsf = dstack[:].rearrange("p k r w -> p k (r w)")
                v._custom_dve(MEDPROD, out=dsf, in0=dsf, in1=medb)
                v.tensor_add(dstack[:, 0:4], dstack[:, 0:4], dstack[:, 4:8])
                v.tensor_add(dstack[:, 0:2], dstack[:, 0:2], dstack[:, 2:4])
                v.tensor_add(dstack[:, 0], dstack[:, 0], dstack[:, 1])
                R = diff[0]

                q = rmed
                v.tensor_tensor(q[:], R[:], rmed[:], ALU.mult)  # q overwrites rmed
                d3 = opool.tile([128, RCH, W], F16, name="d3", tag="d3")
                last = ci == 64 // RCH - 1
                if not last:
                    v.tensor_sub(d3[:], T1[:], q[:])

                # ---- conv2: out = w2a^T d + w2b^T d3 + b2  (block-diagonal)
                # Last chunk: d3/evac/DMA per 4-row sub so the tail pipeline
                # (PE -> ACT -> DMA -> exit barrier) starts immediately.
                osb = opool.tile([128, RCH, W], F32, name="osb", tag="osb")
                for sub in range(RCH // 4):
                    sl = slice(4 * sub, 4 * sub + 4)
                    if last:
                        v.tensor_sub(d3[:, sl], T1[:, sl], q[:, sl])
                    r0 = rr0 + sub * 4
                    ps2 = pp2.tile([128, 4, W], F32, name="ps2", tag="ps2")
                    nc.tensor.matmul(ps2[:], w2asb[:],
                                     dpadE[:, r0:r0 + 4, 1:129],
                                     start=True, stop=False)
                    nc.tensor.matmul(ps2[:], w2bsb[:], d3[:, sl],
                                     start=False, stop=True)
                    s.add(osb[:, sl], ps2[:], b2sb[:])
                    if last:
                        g0 = ci * RCH + 4 * sub
                        nc.sync.dma_start(out[:, g0:g0 + 4, :], osb[0:64, sl])
                        nc.sync.dma_start(out[:, 64 + g0:64 + g0 + 4, :],
                                          osb[64:128, sl])

                if not last:
                    g0 = ci * RCH
                    nc.sync.dma_start(out[:, g0:g0 + RCH, :], osb[0:64])
                    nc.sync.dma_start(out[:, 64 + g0:64 + g0 + RCH, :],
                                      osb[64:128])

    nc.compile()
    return nc


_NC_CACHE = None


def _get_program():
    global _NC_CACHE
    if _NC_CACHE is None:
        _NC_CACHE = build_program()
    return _NC_CACHE


def _host_inputs(x, w1, b1, w2, b2):
    """Build the per-core input maps (shard by batch, prep weights)."""
    f16 = np.float16
    w1t = np.ascontiguousarray(w1.T.astype(f16))            # (c_in, c_out)
    w2at = np.ascontiguousarray(w2[:, :C].T.astype(f16))    # (c, o)
    w2bt = np.ascontiguousarray(w2[:, C:].T.astype(f16))
    bd = lambda m: np.block([[m, np.zeros_like(m)], [np.zeros_like(m), m]]).astype(f16)
    w1bd, w2abd, w2bbd = bd(w1t), bd(w2at), bd(w2bt)
    b1v = np.concatenate([b1, b1]).astype(np.float32).reshape(128, 1)
    b2v = np.concatenate([b2, b2]).astype(np.float32).reshape(128, 1)
    in_maps = []
    for i in range(NCORES):
        in_maps.append({
            "x16": np.ascontiguousarray(x[i].astype(f16)),
            "w1bd": w1bd, "w2abd": w2abd, "w2bbd": w2bbd,
            "b1v": b1v, "b2v": b2v,
        })
    return in_maps


def _spot_check(out, x, w1, b1, w2, b2, b=0, h=5):
    """Host-side reference for one output row; guards against the axon
    relay's rare whole-invocation garbage (seen once: rel ~37 vs 6e-3,
    clean on rerun).  Loose threshold: true fp16 rel_l2 is ~4e-3."""
    d = np.einsum('oc,chw->ohw', w1, x[b, :, h - 1:h + 2, :]) \
        + b1[:, None, None]
    p = np.pad(d, ((0, 0), (0, 0), (1, 1)), mode='reflect')
    nb = np.stack([p[:, i, j:j + 128] for i in range(3) for j in range(3)],
                  axis=-1)
    diff = nb - d[:, 1:2, :].transpose(0, 2, 1)
    absd = np.abs(diff)
    med = np.median(absd, axis=-1, keepdims=True)
    keep = absd <= med
    dz = np.where(keep, absd, 0.0)
    s = dz / np.maximum(dz.max(axis=-1, keepdims=True), 1e-30)
    d3 = (np.where(keep, diff, 0.0) * (1.0 - s)).sum(-1)
    cat = np.concatenate([d[:, 1, :], d3], axis=0)
    ref = np.einsum('oc,cw->ow', w2, cat) + b2[:, None]
    got = out[b, :, h, :]
    denom = np.linalg.norm(ref) + 1e-30
    return np.linalg.norm(got - ref) / denom


def kernel(x, w1, b1, w2, b2):
    x = np.asarray(x, dtype=np.float32)
    w1 = np.asarray(w1, dtype=np.float32)
    b1 = np.asarray(b1, dtype=np.float32)
    w2 = np.asarray(w2, dtype=np.float32)
    b2 = np.asarray(b2, dtype=np.float32)
    nc = _get_program()
    in_maps = _host_inputs(x, w1, b1, w2, b2)
    for attempt in range(3):
        res = run_bass_kernel_spmd(nc, in_maps, core_ids=list(range(NCORES)))
        out = np.stack([res.results[i]["out"] for i in range(NCORES)], axis=0)
        if _spot_check(out, x, w1, b1, w2, b2) < 0.05:
            break
    return out.astype(np.float32)



# revision 10
# speedup vs baseline: 1.0889x; 1.0889x over previous
"""Trainium2 Bass kernel for nn_ASCGM_30090540876360 (3x3 median-trimmed residual
between two 1x1 convs).

Math: reference computes, per (b,c,h,w), over the 9-point reflect-padded
neighborhood of d = conv1x1(x):
    diff_k = n_k - c ; absd_k = |diff_k| ; med = median9(absd)
    keep absd<=med, s = absd/max(kept absd); d3 = sum(diff*(1-s))
Since the center diff is always 0, med = 4th-smallest of the 8 neighbor
|diffs|, max(kept absd) = med, and elements with absd == med contribute 0.
Therefore exactly:
    d3 = T1 - R/med,  T1 = sum_k diff_k,  R = sum_k diff_k * min(absd_k, med)
A tiny floor on the median (med = max(med, 1e-4)) keeps the formula finite
and correct when fp16 rounding creates >=4 zero diffs (reflected corners):
there d3 degrades gracefully to T1 - sum(nonzero diffs) = 0, matching the
reference's behavior at such near-degenerate pixels.

Sharding: data-parallel over batch B=8 across the 8 NeuronCores (1 image per
core).  On-core layout: 128 partitions = 2 image halves x 64 channels; each
partition holds 64 rows (+1 halo row each side) of one half.  Both halves are
processed by single instructions via block-diagonal conv weights.

Stencil runs in fp16 (DVE 2x mode) against a dual-copy padded d buffer
(dpadE / dpadO shifted by one element) so every strided fp16 operand stays
4-byte aligned.
"""
import sys, os
sys.path.insert(0, '/opt/trn_rl_repo')

import numpy as np
from contextlib import ExitStack

import concourse.bass as bass
import concourse.tile as tile
from concourse import bacc, mybir
from concourse.bass_utils import run_bass_kernel_spmd
from concourse import dve_ops as _dve_ops
from concourse.dve_spec import (Spec, Src0, Src1, Zero, One, maxx, minn, lower,
                                scan, AluOp)
from concourse.dve_spec import _has_src1 as has_src1
from concourse.dve_uop import DveOpSpec


def _register_medscan():
    """Custom DVE op: out = cumsum(in0 * min(|in0|*in1, 1)) along the stream.

    Streamed with k innermost per pixel, the per-pixel R/med = sum_k
    diff_k*min(|diff_k|*rmed, 1) falls out as the difference of prefix
    values at consecutive pixels' k=7 slots.  Fuses the product, the
    clip, and the 8-way reduction into one pass."""
    name = "ANT_MEDSCAN_K"
    for op in _dve_ops.OPS:
        if op.name == name:
            return op
    import numpy as _np
    t = Src0 * minn(maxx(Src0, Zero - Src0) * Src1, One)

    def _ref(in0, in1, *a):
        sh = in0.shape
        x = _np.asarray(in0, _np.float64).reshape(sh[0], -1)
        m = _np.asarray(in1, _np.float64)
        m = (m.reshape(x.shape) if m.size == x.size
             else _np.broadcast_to(m.reshape(m.shape[0], -1), x.shape))
        tt = x * _np.minimum(_np.abs(x) * m, 1.0)
        return _np.cumsum(tt, axis=1).reshape(sh)

    spec = Spec(body=scan(AluOp.ADD, t), reference=_ref)
    shas = {}
    op = _dve_ops.DveOp(name, spec, subdim=False, uops_sha=shas)
    _dve_ops.OPS.append(op)
    _dve_ops._SUB_OPCODE_FOR_NAME[name] = (_dve_ops._CUSTOM_DVE_ROW_BASE
                                           + len(_dve_ops.OPS) - 1)
    _dve_ops.CUSTOM_DVE_SPECS[name] = spec
    for ver in ("v3", "v4"):
        r = DveOpSpec(name=name, opcode=_dve_ops.get_dve_sub_opcode(name),
                      uops=lower(spec, ver=ver), rd1_en=has_src1(spec))
        shas[ver] = r.sha(ver)
    return op


MEDSCAN = _register_medscan()

F16 = mybir.dt.float16
F32 = mybir.dt.float32
ALU = mybir.AluOpType
AFT = mybir.ActivationFunctionType

C = 64          # channels
H = W = 128     # image size
NCORES = 8
PR = 66         # padded rows per half (64 + halo)
WP = 130        # padded row width
ETA = 1e-4      # median floor (fp16-safe; see module docstring)
RCH = 16        # stencil chunk rows (per half) -> 4 chunks
NB8 = [(0, 0), (0, 2), (2, 0), (2, 2), (1, 0), (1, 2), (0, 1), (2, 1)]


def build_program():
    nc = bacc.Bacc("TRN2", target_bir_lowering=False, debug=False)

    x16 = nc.dram_tensor("x16", [C, H, W], F16, kind="ExternalInput")
    w1bd = nc.dram_tensor("w1bd", [128, 128], F16, kind="ExternalInput")
    w2abd = nc.dram_tensor("w2abd", [128, 128], F16, kind="ExternalInput")
    w2bbd = nc.dram_tensor("w2bbd", [128, 128], F16, kind="ExternalInput")
    w2bnbd = nc.dram_tensor("w2bnbd", [128, 128], F16, kind="ExternalInput")
    w1n9bd = nc.dram_tensor("w1n9bd", [128, 128], F16, kind="ExternalInput")
    identd = nc.dram_tensor("identd", [128, 128], F16, kind="ExternalInput")
    b1v = nc.dram_tensor("b1v", [128, 1], F32, kind="ExternalInput")
    b2v = nc.dram_tensor("b2v", [128, 1], F32, kind="ExternalInput")
    out = nc.dram_tensor("out", [C, H, W], F32, kind="ExternalOutput")

    v = nc.vector
    s = nc.scalar

    with tile.TileContext(nc) as tc:
        with ExitStack() as ctx:
            cpool = ctx.enter_context(tc.tile_pool(name="const", bufs=1))
            w1sb = cpool.tile([128, 128], F16, tag="w1sb")
            w2asb = cpool.tile([128, 128], F16, tag="w2asb")
            w2bsb = cpool.tile([128, 128], F16, tag="w2bsb")
            w2bnsb = cpool.tile([128, 128], F16, tag="w2bnsb")
            w1n9sb = cpool.tile([128, 128], F16, tag="w1n9sb")
            identsb = cpool.tile([128, 128], F16, tag="identsb")
            b1sb = cpool.tile([128, 1], F32, tag="b1sb")
            b2sb = cpool.tile([128, 1], F32, tag="b2sb")

            dpool = ctx.enter_context(tc.tile_pool(name="dpad", bufs=1))
            dpadE = dpool.tile([128, PR, WP], F16, tag="dpadE")
            dpadO = dpool.tile([128, PR, WP], F16, tag="dpadO")
            dvpad = dpool.tile([128, 64, WP], F16, tag="dvpad")
            b1x3 = cpool.tile([128, 1], F32, name="b1x3", tag="b1x3")
            b1n9 = cpool.tile([128, 1], F32, name="b1n9", tag="b1n9")
            # prefix-scan output buffers (double-buffered across 4-row
            # groups); row 0 stays zero so q[0] = S[0] - 0 works uniformly
            Stiles = [dpool.tile([128, 513, 8], F32, name=f"scanS{i}",
                                 tag=f"scanS{i}") for i in range(2)]

            # ---- load x with halo rows (reflection handled by duplicate DMAs)
            xpool = ctx.enter_context(tc.tile_pool(name="xp", bufs=1))
            xsb = xpool.tile([128, PR, W], F16, tag="xsb")
            # half A: global rows -1..64 -> local 0..65 (row -1 == row 1)
            # half B: global rows 63..128 -> local 0..65 (row 128 == row 126)
            # First pieces small (conv chunk 0 needs only local rows 0..5);
            # the bulk rides the separate SWDGE queue so issues overlap.
            # issue order = critical path: the x rows and w1 gate the first
            # matmul (w1's transfer is tiny; b1 is only needed by the first
            # evacuation, slightly later); everything else after
            nc.sync.dma_start(xsb[0:64, 1:20, :], x16[:, 0:19, :])
            nc.sync.dma_start(xsb[64:128, 0:20, :], x16[:, 63:83, :])
            nc.sync.dma_start(xsb[0:64, 0:1, :], x16[:, 1:2, :])
            nc.sync.dma_start(w1sb[:], w1bd[:])
            nc.sync.dma_start(b1sb[:], b1v[:])
            s.mul(b1x3[:], b1sb[:], 3.0)   # on ACT: keeps DVE free at start
            s.mul(b1n9[:], b1sb[:], -9.0)
            v.memset(Stiles[0][:, 0:1, :], 0.0)
            v.memset(Stiles[1][:, 0:1, :], 0.0)
            nc.sync.dma_start(w2asb[:], w2abd[:])
            nc.sync.dma_start(w2bsb[:], w2bbd[:])
            nc.sync.dma_start(w2bnsb[:], w2bnbd[:])
            nc.sync.dma_start(w1n9sb[:], w1n9bd[:])
            nc.sync.dma_start(identsb[:], identd[:])
            nc.sync.dma_start(b2sb[:], b2v[:])
            nc.sync.dma_start(xsb[64:128, 65:66, :], x16[:, 126:127, :])
            for rr in range(19, 65, 16):  # bulk loads, alternating queues
                ra = min(rr + 16, 65)   # half A: local 1+rr <- global rr
                rb = min(rr + 16, 64)   # half B: local 1+rr <- global 64+rr
                nc.gpsimd.dma_start(xsb[0:64, 1 + rr:1 + ra, :],
                                    x16[:, rr:ra, :])
                if rb > rr:
                    nc.sync.dma_start(xsb[64:128, 1 + rr:1 + rb, :],
                                      x16[:, 64 + rr:64 + rb, :])

            # ---- conv1 producers (emitted per consumer chunk, see loop)
            pp1 = ctx.enter_context(tc.tile_pool(name="psum1", bufs=2,
                                                 space="PSUM"))
            xflat = xsb[:].rearrange("p r w -> p (r w)")
            NTOT = PR * W  # 8448

            def conv1_chunk(cc, e_on_dve=False):
                # conv1 of padded rows 4cc..4cc+3 (last chunk: 2 rows)
                n0 = 512 * cc
                nsz = min(512, NTOT - n0)
                nr = nsz // W
                r0 = 4 * cc
                ps = pp1.tile([128, nr, W], F32, name="ps1", tag="ps1")
                nc.tensor.matmul(ps[:], w1sb[:], xflat[:, n0:n0 + nsz],
                                 start=True, stop=True)
                # evacuate with bias, fp32->fp16, into both shifted pads,
                # including the reflected column pads straight from PSUM
                s.add(dpadE[:, r0:r0 + nr, 1:129], ps[:], b1sb[:])
                if e_on_dve:
                    # prologue only: DVE is idle, so build the odd-shifted
                    # copy from dpadE there and shorten ACT's critical path;
                    # column pads for these rows are emitted afterwards from
                    # dpadE (not PSUM) so they neither occupy ACT's queue
                    # between the gating E-evacuations nor hold PSUM slots
                    v.tensor_copy(dpadO[:, r0:r0 + nr, 2:130],
                                  dpadE[:, r0:r0 + nr, 1:129])
                else:
                    s.add(dpadO[:, r0:r0 + nr, 2:130], ps[:], b1sb[:])
                    s.add(dpadE[:, r0:r0 + nr, 0:130:129],
                          ps[:, :, 1:127:125], b1sb[:])

            def dv_chunk(ch):
                # dv = vertical-3-sum of d (PSUM accumulation over row-shifted
                # rhs views); interior rows 4ch..4ch+3
                m0 = W + 512 * ch
                psv = pp1.tile([128, 4, W], F32, name="psv", tag="psv")
                nc.tensor.matmul(psv[:], w1sb[:],
                                 xflat[:, m0 - W:m0 - W + 512],
                                 start=True, stop=False)
                nc.tensor.matmul(psv[:], w1sb[:], xflat[:, m0:m0 + 512],
                                 start=False, stop=False)
                nc.tensor.matmul(psv[:], w1sb[:],
                                 xflat[:, m0 + W:m0 + W + 512],
                                 start=False, stop=True)
                s.add(dvpad[:, 4 * ch:4 * ch + 4, 1:129], psv[:], b1x3[:])
                s.add(dvpad[:, 4 * ch:4 * ch + 4, 0:130:129],
                      psv[:, :, 1:127:125], b1x3[:])

            # conv1 chunk ranges produced right before the stencil chunk that
            # first needs them (software pipelining via emission order)
            CONV_RANGES = [(0, 5), (5, 9), (9, 13), (13, 17)]
            DV_RANGES = [(0, 4), (4, 8), (8, 12), (12, 16)]

            # ---- stencil + conv2, chunked over rows
            spool = ctx.enter_context(tc.tile_pool(name="sten", bufs=1))
            opool = ctx.enter_context(tc.tile_pool(name="outp", bufs=1))
            pp2 = ctx.enter_context(tc.tile_pool(name="psum2", bufs=2, space="PSUM"))

            def t1_group(ch, T1sb, g):
                # T1 = box9(d) - 9*center, fully on PE: horizontal 3-sum of
                # dvpad via identity matmuls + a -9*w1 center matmul; the
                # bias terms cancel up to the -9*b1 applied at evacuation.
                pst = pp1.tile([128, 4, W], F32, name="pst", tag="pst")
                r4 = 4 * ch
                nc.tensor.matmul(pst[:], identsb[:], dvpad[:, r4:r4 + 4, 0:128],
                                 start=True, stop=False)
                nc.tensor.matmul(pst[:], identsb[:], dvpad[:, r4:r4 + 4, 1:129],
                                 start=False, stop=False)
                nc.tensor.matmul(pst[:], identsb[:], dvpad[:, r4:r4 + 4, 2:130],
                                 start=False, stop=False)
                m0 = W + 512 * ch
                nc.tensor.matmul(pst[:], w1n9sb[:], xflat[:, m0:m0 + 512],
                                 start=False, stop=True)
                s.add(T1sb[:, 4 * g:4 * g + 4, :], pst[:], b1n9[:])

            def nb_view(i, j, r0, nr):
                # neighborhood view (i,j) for chunk local padded rows r0..r0+nr
                if j == 1:
                    return dpadO[:, r0 + i:r0 + i + nr, 2:130]
                return dpadE[:, r0 + i:r0 + i + nr, j:j + 128]

            for ci in range(64 // RCH):
                # conv chunks first: they gate the stencil diffs; dv only
                # feeds T1 which is consumed late in the chunk
                for cc in range(*CONV_RANGES[ci]):
                    conv1_chunk(cc, e_on_dve=(ci == 0))
                if ci == 0:
                    # deferred prologue column pads, sourced from dpadE
                    c0, c1_ = CONV_RANGES[0]
                    s.copy(dpadE[:, 4 * c0:4 * c1_, 0:130:129],
                           dpadE[:, 4 * c0:4 * c1_, 2:128:125])
                rr0 = 1 + ci * RCH          # first interior padded row of chunk
                ctr = dpadO[:, rr0:rr0 + RCH, 2:130]

                def newt(tag, dt=F16):
                    return spool.tile([128, RCH, W], dt, name=tag, tag=tag)

                T1sb = newt("T1sb")
                for ch in range(*DV_RANGES[ci]):
                    dv_chunk(ch)
                    t1_group(ch, T1sb, ch - 4 * ci)

                dstack = spool.tile([128, 8, RCH, W], F16, name="dstack",
                                    tag="dstack")
                diff = [dstack[:, k] for k in range(8)]
                # 4 fused subtracts, 2 neighbors each (k-dim in the AP); the
                # center operand broadcasts over k with a 0-stride dim.
                # For the first chunk, emit in two row-halves so the first
                # half's diffs start as soon as the first conv chunks land.
                eflat = dpadE[:].rearrange("p r w -> p (r w)")
                oflat = dpadO[:].rearrange("p r w -> p (r w)")
                pairs = [  # (src flat view, base row offset, k step)
                    (eflat, rr0 - 1, 0, 2),        # (0,0),(0,2)
                    (eflat, rr0 + 1, 0, 2),        # (2,0),(2,2)
                    (eflat, rr0, 0, 2),            # (1,0),(1,2)
                    (oflat, rr0 - 1, 2, 2 * WP),   # (0,1),(2,1)
                ]
                halves = [(0, RCH // 2), (RCH // 2, RCH - RCH // 2)] \
                    if ci == 0 else [(0, RCH)]
                for hr0, hnr in halves:
                    ctrb = bass.AP(tensor=oflat.tensor,
                                   offset=(rr0 + hr0) * WP + 2,
                                   ap=[oflat.ap[0], [0, 2], [WP, hnr], [1, W]])
                    for pi, (src, brow, bcol, kst) in enumerate(pairs):
                        nb2 = bass.AP(tensor=src.tensor,
                                      offset=(brow + hr0) * WP + bcol,
                                      ap=[src.ap[0], [kst, 2], [WP, hnr],
                                          [1, W]])
                        v.tensor_tensor(dstack[:, 2 * pi:2 * pi + 2,
                                               hr0:hr0 + hnr], nb2, ctrb,
                                        ALU.subtract)


                absd = []
                for k in range(8):
                    ak = newt(f"absd{k}")
                    s.activation(ak[:], diff[k][:], AFT.Abs)   # ACT (DVE offload)
                    absd.append(ak)

                def tt(op, a, b, o):
                    v.tensor_tensor(o[:], a[:], b[:], op)
                    return o

                # 25-op selection of 4th-smallest-of-8 via median-of-9
                # (the 9th value is the always-zero center diff):
                # med9 = med3( max3(mins), med3(mids), min3(maxs) ) over three
                # sorted triples T0=(0,a0,a1), T1=(a2,a3,a4), T2=(a5,a6,a7).
                # absd slots are reused as scratch once their value is dead.
                A = absd
                p0, q0 = newt("p0"), newt("q0")
                tt(ALU.min, A[0], A[1], p0); tt(ALU.max, A[0], A[1], q0)
                # sort3 of (A2,A3,A4) -> lo1=A4, mi1=A0, hi1=A2
                tt(ALU.min, A[2], A[3], A[0]); tt(ALU.max, A[2], A[3], A[1])
                tt(ALU.max, A[1], A[4], A[2]); tt(ALU.min, A[1], A[4], A[3])
                tt(ALU.min, A[0], A[3], A[4]); tt(ALU.max, A[0], A[3], A[0])
                # sort3 of (A5,A6,A7) -> lo2=A7, mi2=A1, hi2=A5
                tt(ALU.min, A[5], A[6], A[1]); tt(ALU.max, A[5], A[6], A[3])
                tt(ALU.max, A[3], A[7], A[5]); tt(ALU.min, A[3], A[7], A[6])
                tt(ALU.min, A[1], A[6], A[7]); tt(ALU.max, A[1], A[6], A[1])
                # combine
                tt(ALU.max, A[4], A[7], A[3])          # mxlo
                tt(ALU.min, A[2], A[5], A[6])          # min(hi1,hi2)
                tt(ALU.min, A[6], q0, A[6])            # mnhi
                tt(ALU.min, p0, A[0], A[4])            # m_ab
                tt(ALU.max, p0, A[0], A[2])            # M_ab
                tt(ALU.min, A[2], A[1], A[7])          # m_c
                tt(ALU.max, A[4], A[7], A[4])          # mdmid
                tt(ALU.min, A[3], A[4], A[0])          # f_ab
                tt(ALU.max, A[3], A[4], A[1])          # F_ab
                tt(ALU.min, A[1], A[6], A[2])          # f_c
                tt(ALU.max, A[0], A[2], A[5])          # med (pre-floor) in A5
                v3 = A[5]
                med = newt("med")
                v.tensor_scalar_max(med[:], v3[:], ETA)  # floor (v3 >= 0)

                medf = newt("medf", F32)
                s.copy(medf[:], med[:])                    # ACT cast f16->f32
                v.reciprocal_approx_fast(medf[:], medf[:])  # in-place 1/med
                rmed = newt("rmed")
                s.copy(rmed[:], medf[:])                   # ACT cast f32->f16

                # q = R/med per pixel via the fused prefix-scan op: stream
                # dstack with k innermost per pixel, accumulate
                # d*min(|d|*rmed, 1); q = difference of consecutive pixels'
                # k=7 prefix slots (row 0 of S stays zero for pixel 0).
                dsf = dstack[:].rearrange("p k r w -> p (k r w)")
                rmf = rmed[:].rearrange("p r w -> p (r w)")
                q = newt("q")
                qf = q[:].rearrange("p r w -> p (r w)")
                last = ci == 64 // RCH - 1

                def scan_group(g):
                    off = 512 * g
                    din = bass.AP(tensor=dsf.tensor, offset=dsf.offset + off,
                                  ap=[dsf.ap[0], [1, 512], [RCH * W, 8]])
                    rin = bass.AP(tensor=rmf.tensor, offset=rmf.offset + off,
                                  ap=[rmf.ap[0], [1, 512], [0, 8]])
                    S = Stiles[(4 * ci + g) % 2]
                    v._custom_dve(MEDSCAN, out=S[:, 1:513, :], in0=din, in1=rin)
                    Sf = S[:].rearrange("p n k -> p (n k)")
                    hi = bass.AP(tensor=Sf.tensor, offset=Sf.offset + 15,
                                 ap=[Sf.ap[0], [8, 512]])
                    lo = bass.AP(tensor=Sf.tensor, offset=Sf.offset + 7,
                                 ap=[Sf.ap[0], [8, 512]])
                    qo = bass.AP(tensor=qf.tensor, offset=qf.offset + off,
                                 ap=[qf.ap[0], [1, 512]])
                    v.tensor_tensor(qo, hi, lo, ALU.subtract)

                # ---- conv2: out = w2a^T d + w2b^T T1 - w2b^T q + b2; the
                # d3 = T1 - q subtraction rides the PSUM accumulation.
                # Per-4-row groups so the tail pipeline (DVE scan -> PE ->
                # ACT -> DMA -> exit barrier) starts immediately.
                osb = opool.tile([128, RCH, W], F32, name="osb", tag="osb")
                for sub in range(RCH // 4):
                    scan_group(sub)
                    sl = slice(4 * sub, 4 * sub + 4)
                    r0 = rr0 + sub * 4
                    ps2 = pp2.tile([128, 4, W], F32, name="ps2", tag="ps2")
                    nc.tensor.matmul(ps2[:], w2asb[:],
                                     dpadE[:, r0:r0 + 4, 1:129],
                                     start=True, stop=False)
                    nc.tensor.matmul(ps2[:], w2bsb[:], T1sb[:, sl],
                                     start=False, stop=False)
                    nc.tensor.matmul(ps2[:], w2bnsb[:], q[:, sl],
                                     start=False, stop=True)
                    s.add(osb[:, sl], ps2[:], b2sb[:])
                    if last:
                        g0 = ci * RCH + 4 * sub
                        nc.sync.dma_start(out[:, g0:g0 + 4, :], osb[0:64, sl])
                        nc.sync.dma_start(out[:, 64 + g0:64 + g0 + 4, :],
                                          osb[64:128, sl])

                if not last:
                    g0 = ci * RCH
                    nc.sync.dma_start(out[:, g0:g0 + RCH, :], osb[0:64])
                    nc.sync.dma_start(out[:, 64 + g0:64 + g0 + RCH, :],
                                      osb[64:128])

    nc.compile()
    return nc


_NC_CACHE = None


def _get_program():
    global _NC_CACHE
    if _NC_CACHE is None:
        _NC_CACHE = build_program()
    return _NC_CACHE


def _host_inputs(x, w1, b1, w2, b2):
    """Build the per-core input maps (shard by batch, prep weights)."""
    f16 = np.float16
    w1t = np.ascontiguousarray(w1.T)                        # (c_in, c_out)
    w2at = np.ascontiguousarray(w2[:, :C].T)                # (c, o)
    w2bt = np.ascontiguousarray(w2[:, C:].T)
    bd = lambda m: np.block([[m, np.zeros_like(m)], [np.zeros_like(m), m]]).astype(f16)
    w1bd, w2abd, w2bbd = bd(w1t), bd(w2at), bd(w2bt)
    w2bnbd = bd(-w2bt)
    w1n9bd = bd(-9.0 * w1t)
    identbd = np.eye(128, dtype=f16)
    b1v = np.concatenate([b1, b1]).astype(np.float32).reshape(128, 1)
    b2v = np.concatenate([b2, b2]).astype(np.float32).reshape(128, 1)
    in_maps = []
    for i in range(NCORES):
        in_maps.append({
            "x16": np.ascontiguousarray(x[i].astype(f16)),
            "w1bd": w1bd, "w2abd": w2abd, "w2bbd": w2bbd,
            "w2bnbd": w2bnbd, "w1n9bd": w1n9bd, "identd": identbd,
            "b1v": b1v, "b2v": b2v,
        })
    return in_maps


def _spot_check(out, x, w1, b1, w2, b2, b=0, h=5):
    """Host-side reference for one output row; guards against the axon
    relay's rare whole-invocation garbage (seen once: rel ~37 vs 6e-3,
    clean on rerun).  Loose threshold: true fp16 rel_l2 is ~4e-3."""
    d = np.einsum('oc,chw->ohw', w1, x[b, :, h - 1:h + 2, :]) \
        + b1[:, None, None]
    p = np.pad(d, ((0, 0), (0, 0), (1, 1)), mode='reflect')
    nb = np.stack([p[:, i, j:j + 128] for i in range(3) for j in range(3)],
                  axis=-1)
    diff = nb - d[:, 1:2, :].transpose(0, 2, 1)
    absd = np.abs(diff)
    med = np.median(absd, axis=-1, keepdims=True)
    keep = absd <= med
    dz = np.where(keep, absd, 0.0)
    s = dz / np.maximum(dz.max(axis=-1, keepdims=True), 1e-30)
    d3 = (np.where(keep, diff, 0.0) * (1.0 - s)).sum(-1)
    cat = np.concatenate([d[:, 1, :], d3], axis=0)
    ref = np.einsum('oc,cw->ow', w2, cat) + b2[:, None]
    got = out[b, :, h, :]
    denom = np.linalg.norm(ref) + 1e-30
    return np.linalg.norm(got - ref) / denom


def kernel(x, w1, b1, w2, b2):
    x = np.asarray(x, dtype=np.float32)
    w1 = np.asarray(w1, dtype=np.float32)
    b1 = np.asarray(b1, dtype=np.float32)
    w2 = np.asarray(w2, dtype=np.float32)
    b2 = np.asarray(b2, dtype=np.float32)
    nc = _get_program()
    in_maps = _host_inputs(x, w1, b1, w2, b2)
    for attempt in range(3):
        res = run_bass_kernel_spmd(nc, in_maps, core_ids=list(range(NCORES)))
        out = np.stack([res.results[i]["out"] for i in range(NCORES)], axis=0)
        if _spot_check(out, x, w1, b1, w2, b2) < 0.05:
            break
    return out.astype(np.float32)



# revision 15
# speedup vs baseline: 1.1560x; 1.0616x over previous
"""Trainium2 Bass kernel for nn_ASCGM_30090540876360 (3x3 median-trimmed residual
between two 1x1 convs).

Math: reference computes, per (b,c,h,w), over the 9-point reflect-padded
neighborhood of d = conv1x1(x):
    diff_k = n_k - c ; absd_k = |diff_k| ; med = median9(absd)
    keep absd<=med, s = absd/max(kept absd); d3 = sum(diff*(1-s))
Since the center diff is always 0, med = 4th-smallest of the 8 neighbor
|diffs|, max(kept absd) = med, and elements with absd == med contribute 0.
Therefore exactly:
    d3 = T1 - R/med,  T1 = sum_k diff_k,  R = sum_k diff_k * min(absd_k, med)
A tiny floor on the median (med = max(med, 1e-4)) keeps the formula finite
and correct when fp16 rounding creates >=4 zero diffs (reflected corners):
there d3 degrades gracefully to T1 - sum(nonzero diffs) = 0, matching the
reference's behavior at such near-degenerate pixels.

Sharding: data-parallel over batch B=8 across the 8 NeuronCores (1 image per
core).  On-core layout: 128 partitions = 2 image halves x 64 channels; each
partition holds 64 rows (+1 halo row each side) of one half.  Both halves are
processed by single instructions via block-diagonal conv weights.

Stencil runs in fp16 (DVE 2x mode) against a dual-copy padded d buffer
(dpadE / dpadO shifted by one element) so every strided fp16 operand stays
4-byte aligned.
"""
import sys, os
sys.path.insert(0, '/opt/trn_rl_repo')

import numpy as np
from contextlib import ExitStack

import concourse.bass as bass
import concourse.tile as tile
from concourse import bacc, mybir
from concourse.bass_utils import run_bass_kernel_spmd
from concourse import dve_ops as _dve_ops
from concourse.dve_spec import (Spec, Src0, Src1, C0, Zero, One, maxx, minn,
                                lower, scan, AluOp)
from concourse.dve_spec import _has_src1 as has_src1
from concourse.dve_uop import DveOpSpec


def _register_medscan():
    """Custom DVE op: out = cumsum(in0 * min(|in0|*in1, 1)) along the stream.

    Streamed with k innermost per pixel, the per-pixel R/med = sum_k
    diff_k*min(|diff_k|*rmed, 1) falls out as the difference of prefix
    values at consecutive pixels' k=7 slots.  Fuses the product, the
    clip, and the 8-way reduction into one pass."""
    name = "ANT_MEDSCAN_K"
    for op in _dve_ops.OPS:
        if op.name == name:
            return op
    import numpy as _np
    t = Src0 * minn(maxx(Src0, Zero - Src0) * Src1, One)

    def _ref(in0, in1, *a):
        sh = in0.shape
        x = _np.asarray(in0, _np.float64).reshape(sh[0], -1)
        m = _np.asarray(in1, _np.float64)
        m = (m.reshape(x.shape) if m.size == x.size
             else _np.broadcast_to(m.reshape(m.shape[0], -1), x.shape))
        tt = x * _np.minimum(_np.abs(x) * m, 1.0)
        return _np.cumsum(tt, axis=1).reshape(sh)

    spec = Spec(body=scan(AluOp.ADD, t), reference=_ref)
    shas = {}
    op = _dve_ops.DveOp(name, spec, subdim=False, uops_sha=shas)
    _dve_ops.OPS.append(op)
    _dve_ops._SUB_OPCODE_FOR_NAME[name] = (_dve_ops._CUSTOM_DVE_ROW_BASE
                                           + len(_dve_ops.OPS) - 1)
    _dve_ops.CUSTOM_DVE_SPECS[name] = spec
    for ver in ("v3", "v4"):
        r = DveOpSpec(name=name, opcode=_dve_ops.get_dve_sub_opcode(name),
                      uops=lower(spec, ver=ver), rd1_en=has_src1(spec))
        shas[ver] = r.sha(ver)
    return op


MEDSCAN = _register_medscan()


def _register_maxmax_eta():
    """out = max(max(in0, in1), s0) — the median network's final comparator
    fused with the ETA floor, emitting fp32 for the reciprocal directly
    (skips two ACT cast hops that stall the DVE pipeline)."""
    name = "ANT_MAXMAX_ETA"
    for op in _dve_ops.OPS:
        if op.name == name:
            return op
    import numpy as _np

    def _ref(in0, in1, c0, *a):
        return _np.maximum(_np.maximum(in0, in1.reshape(in0.shape)), c0)

    spec = Spec(body=maxx(maxx(Src0, Src1), C0), reference=_ref)
    shas = {}
    op = _dve_ops.DveOp(name, spec, subdim=False, uops_sha=shas)
    _dve_ops.OPS.append(op)
    _dve_ops._SUB_OPCODE_FOR_NAME[name] = (_dve_ops._CUSTOM_DVE_ROW_BASE
                                           + len(_dve_ops.OPS) - 1)
    _dve_ops.CUSTOM_DVE_SPECS[name] = spec
    for ver in ("v3", "v4"):
        r = DveOpSpec(name=name, opcode=_dve_ops.get_dve_sub_opcode(name),
                      uops=lower(spec, ver=ver), rd1_en=has_src1(spec))
        shas[ver] = r.sha(ver)
    return op


MAXMAXETA = _register_maxmax_eta()

F16 = mybir.dt.float16
F32 = mybir.dt.float32
ALU = mybir.AluOpType
AFT = mybir.ActivationFunctionType

C = 64          # channels
H = W = 128     # image size
NCORES = 8
PR = 66         # padded rows per half (64 + halo)
WP = 130        # padded row width
ETA = 1e-4      # median floor (fp16-safe; see module docstring)
RCH = 16        # stencil chunk rows (per half) -> 4 chunks
NB8 = [(0, 0), (0, 2), (2, 0), (2, 2), (1, 0), (1, 2), (0, 1), (2, 1)]


def build_program():
    nc = bacc.Bacc("TRN2", target_bir_lowering=False, debug=False)

    x16 = nc.dram_tensor("x16", [C, H, W], F16, kind="ExternalInput")
    w1bd = nc.dram_tensor("w1bd", [128, 128], F16, kind="ExternalInput")
    w2abd = nc.dram_tensor("w2abd", [128, 128], F16, kind="ExternalInput")
    w2bbd = nc.dram_tensor("w2bbd", [128, 128], F16, kind="ExternalInput")
    w2bnbd = nc.dram_tensor("w2bnbd", [128, 128], F16, kind="ExternalInput")
    w1n9bd = nc.dram_tensor("w1n9bd", [128, 128], F16, kind="ExternalInput")
    identd = nc.dram_tensor("identd", [128, 128], F16, kind="ExternalInput")
    b1v = nc.dram_tensor("b1v", [128, 1], F32, kind="ExternalInput")
    b2v = nc.dram_tensor("b2v", [128, 1], F32, kind="ExternalInput")
    out = nc.dram_tensor("out", [C, H, W], F32, kind="ExternalOutput")

    v = nc.vector
    s = nc.scalar

    with tile.TileContext(nc) as tc:
        with ExitStack() as ctx:
            cpool = ctx.enter_context(tc.tile_pool(name="const", bufs=1))
            w1sb = cpool.tile([128, 128], F16, tag="w1sb")
            w2asb = cpool.tile([128, 128], F16, tag="w2asb")
            w2bsb = cpool.tile([128, 128], F16, tag="w2bsb")
            w2bnsb = cpool.tile([128, 128], F16, tag="w2bnsb")
            w1n9sb = cpool.tile([128, 128], F16, tag="w1n9sb")
            identsb = cpool.tile([128, 128], F16, tag="identsb")
            b1sb = cpool.tile([128, 1], F32, tag="b1sb")
            b2sb = cpool.tile([128, 1], F32, tag="b2sb")

            dpool = ctx.enter_context(tc.tile_pool(name="dpad", bufs=1))
            dpadE = dpool.tile([128, PR, WP], F16, tag="dpadE")
            dpadO = dpool.tile([128, PR, WP], F16, tag="dpadO")
            dvpad = dpool.tile([128, 64, WP], F16, tag="dvpad")
            b1x3 = cpool.tile([128, 1], F32, name="b1x3", tag="b1x3")
            b1n9 = cpool.tile([128, 1], F32, name="b1n9", tag="b1n9")
            # prefix-scan output buffers (double-buffered across 4-row
            # groups); row 0 stays zero so q[0] = S[0] - 0 works uniformly
            Stiles = [dpool.tile([128, 513, 8], F32, name=f"scanS{i}",
                                 tag=f"scanS{i}") for i in range(2)]

            # ---- load x with halo rows (reflection handled by duplicate DMAs)
            xpool = ctx.enter_context(tc.tile_pool(name="xp", bufs=1))
            xsb = xpool.tile([128, PR, W], F16, tag="xsb")
            # half A: global rows -1..64 -> local 0..65 (row -1 == row 1)
            # half B: global rows 63..128 -> local 0..65 (row 128 == row 126)
            # First pieces small (conv chunk 0 needs only local rows 0..5);
            # the bulk rides the separate SWDGE queue so issues overlap.
            # issue order = critical path: the x rows and w1 gate the first
            # matmul (w1's transfer is tiny; b1 is only needed by the first
            # evacuation, slightly later); everything else after
            nc.sync.dma_start(xsb[0:64, 1:20, :], x16[:, 0:19, :])
            nc.sync.dma_start(xsb[64:128, 0:20, :], x16[:, 63:83, :])
            nc.sync.dma_start(xsb[0:64, 0:1, :], x16[:, 1:2, :])
            nc.sync.dma_start(w1sb[:], w1bd[:])
            nc.sync.dma_start(b1sb[:], b1v[:])
            s.mul(b1x3[:], b1sb[:], 3.0)   # on ACT: keeps DVE free at start
            s.mul(b1n9[:], b1sb[:], -9.0)
            v.memset(Stiles[0][:, 0:1, :], 0.0)
            v.memset(Stiles[1][:, 0:1, :], 0.0)
            nc.sync.dma_start(w2asb[:], w2abd[:])
            nc.sync.dma_start(w2bsb[:], w2bbd[:])
            nc.sync.dma_start(w2bnsb[:], w2bnbd[:])
            nc.sync.dma_start(w1n9sb[:], w1n9bd[:])
            nc.sync.dma_start(identsb[:], identd[:])
            nc.sync.dma_start(b2sb[:], b2v[:])
            nc.sync.dma_start(xsb[64:128, 65:66, :], x16[:, 126:127, :])
            for rr in range(19, 65, 16):  # bulk loads, alternating queues
                ra = min(rr + 16, 65)   # half A: local 1+rr <- global rr
                rb = min(rr + 16, 64)   # half B: local 1+rr <- global 64+rr
                nc.gpsimd.dma_start(xsb[0:64, 1 + rr:1 + ra, :],
                                    x16[:, rr:ra, :])
                if rb > rr:
                    nc.sync.dma_start(xsb[64:128, 1 + rr:1 + rb, :],
                                      x16[:, 64 + rr:64 + rb, :])

            # ---- conv1 producers (emitted per consumer chunk, see loop)
            pp1 = ctx.enter_context(tc.tile_pool(name="psum1", bufs=2,
                                                 space="PSUM"))
            xflat = xsb[:].rearrange("p r w -> p (r w)")
            NTOT = PR * W  # 8448

            def conv1_chunk(cc, e_on_dve=False):
                # conv1 of padded rows 4cc..4cc+3 (last chunk: 2 rows)
                n0 = 512 * cc
                nsz = min(512, NTOT - n0)
                nr = nsz // W
                r0 = 4 * cc
                ps = pp1.tile([128, nr, W], F32, name="ps1", tag="ps1")
                nc.tensor.matmul(ps[:], w1sb[:], xflat[:, n0:n0 + nsz],
                                 start=True, stop=True)
                # evacuate with bias, fp32->fp16, into both shifted pads,
                # including the reflected column pads straight from PSUM
                s.add(dpadE[:, r0:r0 + nr, 1:129], ps[:], b1sb[:])
                if e_on_dve:
                    # prologue only: DVE is idle, so build the odd-shifted
                    # copy from dpadE there and shorten ACT's critical path;
                    # column pads for these rows are emitted afterwards from
                    # dpadE (not PSUM) so they neither occupy ACT's queue
                    # between the gating E-evacuations nor hold PSUM slots
                    v.tensor_copy(dpadO[:, r0:r0 + nr, 2:130],
                                  dpadE[:, r0:r0 + nr, 1:129])
                else:
                    s.add(dpadO[:, r0:r0 + nr, 2:130], ps[:], b1sb[:])
                    s.add(dpadE[:, r0:r0 + nr, 0:130:129],
                          ps[:, :, 1:127:125], b1sb[:])

            def dv_chunk(ch):
                # dv = vertical-3-sum of d (PSUM accumulation over row-shifted
                # rhs views); interior rows 4ch..4ch+3
                m0 = W + 512 * ch
                psv = pp1.tile([128, 4, W], F32, name="psv", tag="psv")
                nc.tensor.matmul(psv[:], w1sb[:],
                                 xflat[:, m0 - W:m0 - W + 512],
                                 start=True, stop=False)
                nc.tensor.matmul(psv[:], w1sb[:], xflat[:, m0:m0 + 512],
                                 start=False, stop=False)
                nc.tensor.matmul(psv[:], w1sb[:],
                                 xflat[:, m0 + W:m0 + W + 512],
                                 start=False, stop=True)
                s.add(dvpad[:, 4 * ch:4 * ch + 4, 1:129], psv[:], b1x3[:])
                s.add(dvpad[:, 4 * ch:4 * ch + 4, 0:130:129],
                      psv[:, :, 1:127:125], b1x3[:])

            # conv1 chunk ranges produced right before the stencil chunk that
            # first needs them (software pipelining via emission order)
            CONV_RANGES = [(0, 5), (5, 9), (9, 13), (13, 17)]
            DV_RANGES = [(0, 4), (4, 8), (8, 12), (12, 16)]

            # ---- stencil + conv2, chunked over rows
            spool = ctx.enter_context(tc.tile_pool(name="sten", bufs=1))
            opool = ctx.enter_context(tc.tile_pool(name="outp", bufs=1))
            pp2 = ctx.enter_context(tc.tile_pool(name="psum2", bufs=2, space="PSUM"))

            def t1_group(ch, T1sb, g):
                # T1 = box9(d) - 9*center, fully on PE: horizontal 3-sum of
                # dvpad via identity matmuls + a -9*w1 center matmul; the
                # bias terms cancel up to the -9*b1 applied at evacuation.
                pst = pp1.tile([128, 4, W], F32, name="pst", tag="pst")
                r4 = 4 * ch
                nc.tensor.matmul(pst[:], identsb[:], dvpad[:, r4:r4 + 4, 0:128],
                                 start=True, stop=False)
                nc.tensor.matmul(pst[:], identsb[:], dvpad[:, r4:r4 + 4, 1:129],
                                 start=False, stop=False)
                nc.tensor.matmul(pst[:], identsb[:], dvpad[:, r4:r4 + 4, 2:130],
                                 start=False, stop=False)
                m0 = W + 512 * ch
                nc.tensor.matmul(pst[:], w1n9sb[:], xflat[:, m0:m0 + 512],
                                 start=False, stop=True)
                s.add(T1sb[:, 4 * g:4 * g + 4, :], pst[:], b1n9[:])

            def nb_view(i, j, r0, nr):
                # neighborhood view (i,j) for chunk local padded rows r0..r0+nr
                if j == 1:
                    return dpadO[:, r0 + i:r0 + i + nr, 2:130]
                return dpadE[:, r0 + i:r0 + i + nr, j:j + 128]

            for ci in range(64 // RCH):
                # conv chunks first: they gate the stencil diffs; dv only
                # feeds T1 which is consumed late in the chunk
                for cc in range(*CONV_RANGES[ci]):
                    conv1_chunk(cc, e_on_dve=(ci == 0))
                if ci == 0:
                    # deferred prologue column pads, sourced from dpadE
                    c0, c1_ = CONV_RANGES[0]
                    s.copy(dpadE[:, 4 * c0:4 * c1_, 0:130:129],
                           dpadE[:, 4 * c0:4 * c1_, 2:128:125])
                rr0 = 1 + ci * RCH          # first interior padded row of chunk
                ctr = dpadO[:, rr0:rr0 + RCH, 2:130]

                def newt(tag, dt=F16):
                    return spool.tile([128, RCH, W], dt, name=tag, tag=tag)

                T1sb = newt("T1sb")
                for ch in range(*DV_RANGES[ci]):
                    dv_chunk(ch)
                    t1_group(ch, T1sb, ch - 4 * ci)

                dstack = spool.tile([128, 8, RCH, W], F16, name="dstack",
                                    tag="dstack")
                diff = [dstack[:, k] for k in range(8)]
                # 4 fused subtracts, 2 neighbors each (k-dim in the AP); the
                # center operand broadcasts over k with a 0-stride dim.
                # For the first chunk, emit in two row-halves so the first
                # half's diffs start as soon as the first conv chunks land.
                eflat = dpadE[:].rearrange("p r w -> p (r w)")
                oflat = dpadO[:].rearrange("p r w -> p (r w)")
                pairs = [  # (src flat view, base row offset, k step)
                    (eflat, rr0 - 1, 0, 2),        # (0,0),(0,2)
                    (eflat, rr0 + 1, 0, 2),        # (2,0),(2,2)
                    (eflat, rr0, 0, 2),            # (1,0),(1,2)
                    (oflat, rr0 - 1, 2, 2 * WP),   # (0,1),(2,1)
                ]
                halves = [(0, RCH // 2), (RCH // 2, RCH - RCH // 2)] \
                    if ci == 0 else [(0, RCH)]
                for hr0, hnr in halves:
                    ctrb = bass.AP(tensor=oflat.tensor,
                                   offset=(rr0 + hr0) * WP + 2,
                                   ap=[oflat.ap[0], [0, 2], [WP, hnr], [1, W]])
                    for pi, (src, brow, bcol, kst) in enumerate(pairs):
                        nb2 = bass.AP(tensor=src.tensor,
                                      offset=(brow + hr0) * WP + bcol,
                                      ap=[src.ap[0], [kst, 2], [WP, hnr],
                                          [1, W]])
                        v.tensor_tensor(dstack[:, 2 * pi:2 * pi + 2,
                                               hr0:hr0 + hnr], nb2, ctrb,
                                        ALU.subtract)


                absd = []
                for k in range(8):
                    ak = newt(f"absd{k}")
                    s.activation(ak[:], diff[k][:], AFT.Abs)   # ACT (DVE offload)
                    absd.append(ak)

                def tt(op, a, b, o):
                    v.tensor_tensor(o[:], a[:], b[:], op)
                    return o

                # 25-op selection of 4th-smallest-of-8 via median-of-9
                # (the 9th value is the always-zero center diff):
                # med9 = med3( max3(mins), med3(mids), min3(maxs) ) over three
                # sorted triples T0=(0,a0,a1), T1=(a2,a3,a4), T2=(a5,a6,a7).
                # absd slots are reused as scratch once their value is dead.
                A = absd
                p0, q0 = newt("p0"), newt("q0")
                tt(ALU.min, A[0], A[1], p0); tt(ALU.max, A[0], A[1], q0)
                # sort3 of (A2,A3,A4) -> lo1=A4, mi1=A0, hi1=A2
                tt(ALU.min, A[2], A[3], A[0]); tt(ALU.max, A[2], A[3], A[1])
                tt(ALU.max, A[1], A[4], A[2]); tt(ALU.min, A[1], A[4], A[3])
                tt(ALU.min, A[0], A[3], A[4]); tt(ALU.max, A[0], A[3], A[0])
                # sort3 of (A5,A6,A7) -> lo2=A7, mi2=A1, hi2=A5
                tt(ALU.min, A[5], A[6], A[1]); tt(ALU.max, A[5], A[6], A[3])
                tt(ALU.max, A[3], A[7], A[5]); tt(ALU.min, A[3], A[7], A[6])
                tt(ALU.min, A[1], A[6], A[7]); tt(ALU.max, A[1], A[6], A[1])
                # combine
                tt(ALU.max, A[4], A[7], A[3])          # mxlo
                tt(ALU.min, A[2], A[5], A[6])          # min(hi1,hi2)
                tt(ALU.min, A[6], q0, A[6])            # mnhi
                tt(ALU.min, p0, A[0], A[4])            # m_ab
                tt(ALU.max, p0, A[0], A[2])            # M_ab
                tt(ALU.min, A[2], A[1], A[7])          # m_c
                tt(ALU.max, A[4], A[7], A[4])          # mdmid
                tt(ALU.min, A[3], A[4], A[0])          # f_ab
                tt(ALU.max, A[3], A[4], A[1])          # F_ab
                tt(ALU.min, A[1], A[6], A[2])          # f_c
                # final comparator fused with the ETA floor, fp32 out; then
                # reciprocal in place -> medf holds 1/max(med, ETA).  All on
                # DVE: no ACT cast hops on the chunk critical path.
                medf = newt("medf", F32)
                v._custom_dve(MAXMAXETA, out=medf[:], in0=A[0][:], in1=A[2][:],
                              s0=ETA)
                v.reciprocal_approx_fast(medf[:], medf[:])

                # q = R/med per pixel via the fused prefix-scan op: stream
                # dstack with k innermost per pixel, accumulate
                # d*min(|d|*rmed, 1); q = difference of consecutive pixels'
                # k=7 prefix slots (row 0 of S stays zero for pixel 0).
                dsf = dstack[:].rearrange("p k r w -> p (k r w)")
                rmf = medf[:].rearrange("p r w -> p (r w)")
                q = newt("q")
                qf = q[:].rearrange("p r w -> p (r w)")
                last = ci == 64 // RCH - 1

                def scan_group(g):
                    off = 512 * g
                    din = bass.AP(tensor=dsf.tensor, offset=dsf.offset + off,
                                  ap=[dsf.ap[0], [1, 512], [RCH * W, 8]])
                    rin = bass.AP(tensor=rmf.tensor, offset=rmf.offset + off,
                                  ap=[rmf.ap[0], [1, 512], [0, 8]])
                    S = Stiles[(4 * ci + g) % 2]
                    v._custom_dve(MEDSCAN, out=S[:, 1:513, :], in0=din, in1=rin)
                    Sf = S[:].rearrange("p n k -> p (n k)")
                    hi = bass.AP(tensor=Sf.tensor, offset=Sf.offset + 15,
                                 ap=[Sf.ap[0], [8, 512]])
                    lo = bass.AP(tensor=Sf.tensor, offset=Sf.offset + 7,
                                 ap=[Sf.ap[0], [8, 512]])
                    qo = bass.AP(tensor=qf.tensor, offset=qf.offset + off,
                                 ap=[qf.ap[0], [1, 512]])
                    v.tensor_tensor(qo, hi, lo, ALU.subtract)

                # ---- conv2: out = w2a^T d + w2b^T T1 - w2b^T q + b2; the
                # d3 = T1 - q subtraction rides the PSUM accumulation.
                # Per-4-row groups so the tail pipeline (DVE scan -> PE ->
                # ACT -> DMA -> exit barrier) starts immediately.
                osb = opool.tile([128, RCH, W], F32, name="osb", tag="osb")
                for sub in range(RCH // 4):
                    scan_group(sub)
                    sl = slice(4 * sub, 4 * sub + 4)
                    r0 = rr0 + sub * 4
                    ps2 = pp2.tile([128, 4, W], F32, name="ps2", tag="ps2")
                    nc.tensor.matmul(ps2[:], w2asb[:],
                                     dpadE[:, r0:r0 + 4, 1:129],
                                     start=True, stop=False)
                    nc.tensor.matmul(ps2[:], w2bsb[:], T1sb[:, sl],
                                     start=False, stop=False)
                    nc.tensor.matmul(ps2[:], w2bnsb[:], q[:, sl],
                                     start=False, stop=True)
                    s.add(osb[:, sl], ps2[:], b2sb[:])
                    if last:
                        g0 = ci * RCH + 4 * sub
                        nc.sync.dma_start(out[:, g0:g0 + 4, :], osb[0:64, sl])
                        nc.sync.dma_start(out[:, 64 + g0:64 + g0 + 4, :],
                                          osb[64:128, sl])

                if not last:
                    g0 = ci * RCH
                    nc.sync.dma_start(out[:, g0:g0 + RCH, :], osb[0:64])
                    nc.sync.dma_start(out[:, 64 + g0:64 + g0 + RCH, :],
                                      osb[64:128])

    nc.compile()
    return nc


_NC_CACHE = None


def _get_program():
    global _NC_CACHE
    if _NC_CACHE is None:
        _NC_CACHE = build_program()
    return _NC_CACHE


def _host_inputs(x, w1, b1, w2, b2):
    """Build the per-core input maps (shard by batch, prep weights)."""
    f16 = np.float16
    w1t = np.ascontiguousarray(w1.T)                        # (c_in, c_out)
    w2at = np.ascontiguousarray(w2[:, :C].T)                # (c, o)
    w2bt = np.ascontiguousarray(w2[:, C:].T)
    bd = lambda m: np.block([[m, np.zeros_like(m)], [np.zeros_like(m), m]]).astype(f16)
    w1bd, w2abd, w2bbd = bd(w1t), bd(w2at), bd(w2bt)
    w2bnbd = bd(-w2bt)
    w1n9bd = bd(-9.0 * w1t)
    identbd = np.eye(128, dtype=f16)
    b1v = np.concatenate([b1, b1]).astype(np.float32).reshape(128, 1)
    b2v = np.concatenate([b2, b2]).astype(np.float32).reshape(128, 1)
    in_maps = []
    for i in range(NCORES):
        in_maps.append({
            "x16": np.ascontiguousarray(x[i].astype(f16)),
            "w1bd": w1bd, "w2abd": w2abd, "w2bbd": w2bbd,
            "w2bnbd": w2bnbd, "w1n9bd": w1n9bd, "identd": identbd,
            "b1v": b1v, "b2v": b2v,
        })
    return in_maps


def _spot_check(out, x, w1, b1, w2, b2, b=0, h=5):
    """Host-side reference for one output row; guards against the axon
    relay's rare whole-invocation garbage (seen once: rel ~37 vs 6e-3,
    clean on rerun).  Loose threshold: true fp16 rel_l2 is ~4e-3."""
    d = np.einsum('oc,chw->ohw', w1, x[b, :, h - 1:h + 2, :]) \
        + b1[:, None, None]
    p = np.pad(d, ((0, 0), (0, 0), (1, 1)), mode='reflect')
    nb = np.stack([p[:, i, j:j + 128] for i in range(3) for j in range(3)],
                  axis=-1)
    diff = nb - d[:, 1:2, :].transpose(0, 2, 1)
    absd = np.abs(diff)
    med = np.median(absd, axis=-1, keepdims=True)
    keep = absd <= med
    dz = np.where(keep, absd, 0.0)
    s = dz / np.maximum(dz.max(axis=-1, keepdims=True), 1e-30)
    d3 = (np.where(keep, diff, 0.0) * (1.0 - s)).sum(-1)
    cat = np.concatenate([d[:, 1, :], d3], axis=0)
    ref = np.einsum('oc,cw->ow', w2, cat) + b2[:, None]
    got = out[b, :, h, :]
    denom = np.linalg.norm(ref) + 1e-30
    return np.linalg.norm(got - ref) / denom


def kernel(x, w1, b1, w2, b2):
    x = np.asarray(x, dtype=np.float32)
    w1 = np.asarray(w1, dtype=np.float32)
    b1 = np.asarray(b1, dtype=np.float32)
    w2 = np.asarray(w2, dtype=np.float32)
    b2 = np.asarray(b2, dtype=np.float32)
    nc = _get_program()
    in_maps = _host_inputs(x, w1, b1, w2, b2)
    for attempt in range(3):
        res = run_bass_kernel_spmd(nc, in_maps, core_ids=list(range(NCORES)))
        out = np.stack([res.results[i]["out"] for i in range(NCORES)], axis=0)
        if _spot_check(out, x, w1, b1, w2, b2) < 0.05:
            break
    return out.astype(np.float32)



# revision 27
# speedup vs baseline: 1.2121x; 1.0485x over previous
"""Trainium2 Bass kernel for nn_ASCGM_30090540876360 (3x3 median-trimmed residual
between two 1x1 convs).

Math: reference computes, per (b,c,h,w), over the 9-point reflect-padded
neighborhood of d = conv1x1(x):
    diff_k = n_k - c ; absd_k = |diff_k| ; med = median9(absd)
    keep absd<=med, s = absd/max(kept absd); d3 = sum(diff*(1-s))
Since the center diff is always 0, med = 4th-smallest of the 8 neighbor
|diffs|, max(kept absd) = med, and elements with absd == med contribute 0.
Therefore exactly:
    d3 = T1 - R/med,  T1 = sum_k diff_k,  R = sum_k diff_k * min(absd_k, med)
A tiny floor on the median (med = max(med, 1e-4)) keeps the formula finite
and correct when fp16 rounding creates >=4 zero diffs (reflected corners):
there d3 degrades gracefully to T1 - sum(nonzero diffs) = 0, matching the
reference's behavior at such near-degenerate pixels.

Sharding: data-parallel over batch B=8 across the 8 NeuronCores (1 image per
core).  On-core layout: 128 partitions = 2 image halves x 64 channels; each
partition holds 64 rows (+1 halo row each side) of one half.  Both halves are
processed by single instructions via block-diagonal conv weights.

Stencil runs in fp16 (DVE 2x mode) against a dual-copy padded d buffer
(dpadE / dpadO shifted by one element) so every strided fp16 operand stays
4-byte aligned.
"""
import sys, os
sys.path.insert(0, '/opt/trn_rl_repo')

import numpy as np
from contextlib import ExitStack

import concourse.bass as bass
import concourse.tile as tile
from concourse import bacc, mybir
from concourse.bass_utils import run_bass_kernel_spmd
from concourse import dve_ops as _dve_ops
from concourse.dve_spec import (Spec, Src0, Src1, C0, Zero, One, maxx, minn,
                                lower, scan, AluOp)
from concourse.dve_spec import _has_src1 as has_src1
from concourse.dve_uop import DveOpSpec


def _register_medscan():
    """Custom DVE op: out = cumsum(in0 * min(|in0|*in1, 1)) along the stream.

    Streamed with k innermost per pixel, the per-pixel R/med = sum_k
    diff_k*min(|diff_k|*rmed, 1) falls out as the difference of prefix
    values at consecutive pixels' k=7 slots.  Fuses the product, the
    clip, and the 8-way reduction into one pass."""
    name = "ANT_MEDSCAN_K"
    for op in _dve_ops.OPS:
        if op.name == name:
            return op
    import numpy as _np
    t = Src0 * minn(maxx(Src0, Zero - Src0) * Src1, One)

    def _ref(in0, in1, *a):
        sh = in0.shape
        x = _np.asarray(in0, _np.float64).reshape(sh[0], -1)
        m = _np.asarray(in1, _np.float64)
        m = (m.reshape(x.shape) if m.size == x.size
             else _np.broadcast_to(m.reshape(m.shape[0], -1), x.shape))
        tt = x * _np.minimum(_np.abs(x) * m, 1.0)
        return _np.cumsum(tt, axis=1).reshape(sh)

    spec = Spec(body=scan(AluOp.ADD, t), reference=_ref)
    shas = {}
    op = _dve_ops.DveOp(name, spec, subdim=False, uops_sha=shas)
    _dve_ops.OPS.append(op)
    _dve_ops._SUB_OPCODE_FOR_NAME[name] = (_dve_ops._CUSTOM_DVE_ROW_BASE
                                           + len(_dve_ops.OPS) - 1)
    _dve_ops.CUSTOM_DVE_SPECS[name] = spec
    for ver in ("v3", "v4"):
        r = DveOpSpec(name=name, opcode=_dve_ops.get_dve_sub_opcode(name),
                      uops=lower(spec, ver=ver), rd1_en=has_src1(spec))
        shas[ver] = r.sha(ver)
    return op


MEDSCAN = _register_medscan()


def _register_maxmax_eta():
    """out = max(max(in0, in1), s0) — the median network's final comparator
    fused with the ETA floor, emitting fp32 for the reciprocal directly
    (skips two ACT cast hops that stall the DVE pipeline)."""
    name = "ANT_MAXMAX_ETA"
    for op in _dve_ops.OPS:
        if op.name == name:
            return op
    import numpy as _np

    def _ref(in0, in1, c0, *a):
        return _np.maximum(_np.maximum(in0, in1.reshape(in0.shape)), c0)

    spec = Spec(body=maxx(maxx(Src0, Src1), C0), reference=_ref)
    shas = {}
    op = _dve_ops.DveOp(name, spec, subdim=False, uops_sha=shas)
    _dve_ops.OPS.append(op)
    _dve_ops._SUB_OPCODE_FOR_NAME[name] = (_dve_ops._CUSTOM_DVE_ROW_BASE
                                           + len(_dve_ops.OPS) - 1)
    _dve_ops.CUSTOM_DVE_SPECS[name] = spec
    for ver in ("v3", "v4"):
        r = DveOpSpec(name=name, opcode=_dve_ops.get_dve_sub_opcode(name),
                      uops=lower(spec, ver=ver), rd1_en=has_src1(spec))
        shas[ver] = r.sha(ver)
    return op


MAXMAXETA = _register_maxmax_eta()

F16 = mybir.dt.float16
F32 = mybir.dt.float32
ALU = mybir.AluOpType
AFT = mybir.ActivationFunctionType

C = 64          # channels
H = W = 128     # image size
NCORES = 8
PR = 66         # padded rows per half (64 + halo)
WP = 130        # padded row width
ETA = 1e-4      # median floor (fp16-safe; see module docstring)
RCH = 16        # stencil chunk rows (per half) -> 4 chunks
NB8 = [(0, 0), (0, 2), (2, 0), (2, 2), (1, 0), (1, 2), (0, 1), (2, 1)]


def build_program():
    nc = bacc.Bacc("TRN2", target_bir_lowering=False, debug=False)

    x16 = nc.dram_tensor("x16", [C, H, W], F16, kind="ExternalInput")
    w1bd = nc.dram_tensor("w1bd", [128, 128], F16, kind="ExternalInput")
    w2abd = nc.dram_tensor("w2abd", [128, 128], F16, kind="ExternalInput")
    w2bbd = nc.dram_tensor("w2bbd", [128, 128], F16, kind="ExternalInput")
    w2bnbd = nc.dram_tensor("w2bnbd", [128, 128], F16, kind="ExternalInput")
    w2bf = nc.dram_tensor("w2bf", [128, 128], F32, kind="ExternalInput")
    w2bnf = nc.dram_tensor("w2bnf", [128, 128], F32, kind="ExternalInput")
    w1n9bd = nc.dram_tensor("w1n9bd", [128, 128], F16, kind="ExternalInput")
    identd = nc.dram_tensor("identd", [128, 128], F16, kind="ExternalInput")
    b1v = nc.dram_tensor("b1v", [128, 1], F32, kind="ExternalInput")
    b2v = nc.dram_tensor("b2v", [128, 1], F32, kind="ExternalInput")
    out = nc.dram_tensor("out", [C, H, W], F32, kind="ExternalOutput")

    v = nc.vector
    s = nc.scalar

    with tile.TileContext(nc) as tc:
        with ExitStack() as ctx:
            cpool = ctx.enter_context(tc.tile_pool(name="const", bufs=1))
            w1sb = cpool.tile([128, 128], F16, tag="w1sb")
            w2asb = cpool.tile([128, 128], F16, tag="w2asb")
            w2bsb = cpool.tile([128, 128], F16, tag="w2bsb")
            w2bnsb = cpool.tile([128, 128], F16, tag="w2bnsb")
            w2bf32 = cpool.tile([128, 128], F32, tag="w2bf32")
            w2bnf32 = cpool.tile([128, 128], F32, tag="w2bnf32")
            w1n9sb = cpool.tile([128, 128], F16, tag="w1n9sb")
            identsb = cpool.tile([128, 128], F16, tag="identsb")
            b1sb = cpool.tile([128, 1], F32, tag="b1sb")
            b2sb = cpool.tile([128, 1], F32, tag="b2sb")

            dpool = ctx.enter_context(tc.tile_pool(name="dpad", bufs=1))
            dpadE = dpool.tile([128, PR, WP], F16, tag="dpadE")
            dpadO = dpool.tile([128, PR, WP], F16, tag="dpadO")
            dvpad = dpool.tile([128, 64, WP], F16, tag="dvpad")
            b1x3 = cpool.tile([128, 1], F32, name="b1x3", tag="b1x3")
            b1n9 = cpool.tile([128, 1], F32, name="b1n9", tag="b1n9")
            # prefix-scan output buffers (double-buffered across 4-row
            # groups); row 0 stays zero so q[0] = S[0] - 0 works uniformly
            Stiles = [dpool.tile([128, 513, 8], F32, name=f"scanS{i}",
                                 tag=f"scanS{i}") for i in range(2)]

            # ---- load x with halo rows (reflection handled by duplicate DMAs)
            xpool = ctx.enter_context(tc.tile_pool(name="xp", bufs=1))
            xsb = xpool.tile([128, PR, W], F16, tag="xsb")
            # half A: global rows -1..64 -> local 0..65 (row -1 == row 1)
            # half B: global rows 63..128 -> local 0..65 (row 128 == row 126)
            # First pieces small (conv chunk 0 needs only local rows 0..5);
            # the bulk rides the separate SWDGE queue so issues overlap.
            # issue order = critical path: the x rows and w1 gate the first
            # matmul (w1's transfer is tiny; b1 is only needed by the first
            # evacuation, slightly later); everything else after
            nc.gpsimd.dma_start(w1sb[:], w1bd[:])
            nc.gpsimd.dma_start(b1sb[:], b1v[:])
            nc.sync.dma_start(xsb[0:64, 0:1, :], x16[:, 1:2, :])
            nc.sync.dma_start(xsb[0:64, 1:8, :], x16[:, 0:7, :])
            nc.sync.dma_start(xsb[64:128, 0:8, :], x16[:, 63:71, :])
            nc.sync.dma_start(xsb[0:64, 8:20, :], x16[:, 7:19, :])
            nc.sync.dma_start(xsb[64:128, 8:20, :], x16[:, 71:83, :])
            s.mul(b1x3[:], b1sb[:], 3.0)   # on ACT: keeps DVE free at start
            s.mul(b1n9[:], b1sb[:], -9.0)
            v.memset(Stiles[0][:, 0:1, :], 0.0)
            v.memset(Stiles[1][:, 0:1, :], 0.0)
            nc.sync.dma_start(w2asb[:], w2abd[:])
            nc.sync.dma_start(w2bsb[:], w2bbd[:])
            nc.sync.dma_start(w2bnsb[:], w2bnbd[:])
            nc.sync.dma_start(w2bf32[:], w2bf[:])
            nc.sync.dma_start(w2bnf32[:], w2bnf[:])
            nc.sync.dma_start(w1n9sb[:], w1n9bd[:])
            nc.sync.dma_start(identsb[:], identd[:])
            nc.sync.dma_start(b2sb[:], b2v[:])
            nc.sync.dma_start(xsb[64:128, 65:66, :], x16[:, 126:127, :])
            for rr in range(19, 65, 16):  # bulk loads, alternating queues
                ra = min(rr + 16, 65)   # half A: local 1+rr <- global rr
                rb = min(rr + 16, 64)   # half B: local 1+rr <- global 64+rr
                nc.gpsimd.dma_start(xsb[0:64, 1 + rr:1 + ra, :],
                                    x16[:, rr:ra, :])
                if rb > rr:
                    nc.sync.dma_start(xsb[64:128, 1 + rr:1 + rb, :],
                                      x16[:, 64 + rr:64 + rb, :])

            # ---- conv1 producers (emitted per consumer chunk, see loop)
            pp1 = ctx.enter_context(tc.tile_pool(name="psum1", bufs=2,
                                                 space="PSUM"))
            xflat = xsb[:].rearrange("p r w -> p (r w)")
            NTOT = PR * W  # 8448

            def conv1_chunk(r0, nr, e_on_dve=False):
                # conv1 of padded rows r0..r0+nr
                n0 = W * r0
                nsz = W * nr
                ps = pp1.tile([128, nr, W], F32, name="ps1", tag="ps1")
                nc.tensor.matmul(ps[:], w1sb[:], xflat[:, n0:n0 + nsz],
                                 start=True, stop=True)
                # evacuate with bias, fp32->fp16, into both shifted pads,
                # including the reflected column pads straight from PSUM
                s.add(dpadE[:, r0:r0 + nr, 1:129], ps[:], b1sb[:])
                if e_on_dve:
                    # prologue only: DVE is idle, so build the odd-shifted
                    # copy from dpadE there and shorten ACT's critical path;
                    # column pads for these rows are emitted afterwards from
                    # dpadE (not PSUM) so they neither occupy ACT's queue
                    # between the gating E-evacuations nor hold PSUM slots
                    v.tensor_copy(dpadO[:, r0:r0 + nr, 2:130],
                                  dpadE[:, r0:r0 + nr, 1:129])
                else:
                    s.add(dpadO[:, r0:r0 + nr, 2:130], ps[:], b1sb[:])
                    s.add(dpadE[:, r0:r0 + nr, 0:130:129],
                          ps[:, :, 1:127:125], b1sb[:])

            def dv_chunk(ch):
                # dv = vertical-3-sum of d (PSUM accumulation over row-shifted
                # rhs views); interior rows 4ch..4ch+3
                m0 = W + 512 * ch
                psv = pp1.tile([128, 4, W], F32, name="psv", tag="psv")
                nc.tensor.matmul(psv[:], w1sb[:],
                                 xflat[:, m0 - W:m0 - W + 512],
                                 start=True, stop=False)
                nc.tensor.matmul(psv[:], w1sb[:], xflat[:, m0:m0 + 512],
                                 start=False, stop=False)
                nc.tensor.matmul(psv[:], w1sb[:],
                                 xflat[:, m0 + W:m0 + W + 512],
                                 start=False, stop=True)
                s.add(dvpad[:, 4 * ch:4 * ch + 4, 1:129], psv[:], b1x3[:])
                s.add(dvpad[:, 4 * ch:4 * ch + 4, 0:130:129],
                      psv[:, :, 1:127:125], b1x3[:])

            # conv1 row-range pieces produced right before the stencil chunk
            # that first needs them (software pipelining via emission order);
            # the prologue ladder uses 2-row pieces so the first diffs start
            # as early as possible
            CONV_PIECES = [
                [(0, 2), (2, 2), (4, 2), (6, 2), (8, 4), (12, 4), (16, 4)],
                [(20, 4), (24, 4), (28, 4), (32, 4)],
                [(36, 4), (40, 4), (44, 4), (48, 4)],
                [(52, 4), (56, 4), (60, 4), (64, 2)],
            ]
            DV_RANGES = [(0, 4), (4, 8), (8, 12), (12, 16)]

            # ---- stencil + conv2, chunked over rows
            spool = ctx.enter_context(tc.tile_pool(name="sten", bufs=1))
            opool = ctx.enter_context(tc.tile_pool(name="outp", bufs=1))
            pp2 = ctx.enter_context(tc.tile_pool(name="psum2", bufs=2, space="PSUM"))

            def t1_group(ch, T1sb, g):
                # T1 = box9(d) - 9*center, fully on PE: horizontal 3-sum of
                # dvpad via identity matmuls + a -9*w1 center matmul; the
                # bias terms cancel up to the -9*b1 applied at evacuation.
                pst = pp1.tile([128, 4, W], F32, name="pst", tag="pst")
                r4 = 4 * ch
                nc.tensor.matmul(pst[:], identsb[:], dvpad[:, r4:r4 + 4, 0:128],
                                 start=True, stop=False)
                nc.tensor.matmul(pst[:], identsb[:], dvpad[:, r4:r4 + 4, 1:129],
                                 start=False, stop=False)
                nc.tensor.matmul(pst[:], identsb[:], dvpad[:, r4:r4 + 4, 2:130],
                                 start=False, stop=False)
                m0 = W + 512 * ch
                nc.tensor.matmul(pst[:], w1n9sb[:], xflat[:, m0:m0 + 512],
                                 start=False, stop=True)
                s.add(T1sb[:, 4 * g:4 * g + 4, :], pst[:], b1n9[:])

            def nb_view(i, j, r0, nr):
                # neighborhood view (i,j) for chunk local padded rows r0..r0+nr
                if j == 1:
                    return dpadO[:, r0 + i:r0 + i + nr, 2:130]
                return dpadE[:, r0 + i:r0 + i + nr, j:j + 128]

            for ci in range(64 // RCH):
                # conv chunks first: they gate the stencil diffs; dv only
                # feeds T1 which is consumed late in the chunk
                for pr0, pnr in CONV_PIECES[ci]:
                    conv1_chunk(pr0, pnr, e_on_dve=(ci == 0))
                    if ci == 0 and pr0 + pnr == 12:
                        # deferred prologue column pads in two pieces so the
                        # first half's diffs aren't gated on later conv pieces
                        s.copy(dpadE[:, 0:12, 0:130:129],
                               dpadE[:, 0:12, 2:128:125])
                if ci == 0:
                    s.copy(dpadE[:, 12:20, 0:130:129],
                           dpadE[:, 12:20, 2:128:125])
                rr0 = 1 + ci * RCH          # first interior padded row of chunk
                ctr = dpadO[:, rr0:rr0 + RCH, 2:130]

                def newt(tag, dt=F16):
                    return spool.tile([128, RCH, W], dt, name=tag, tag=tag)

                T1sb = newt("T1sb")

                dstack = spool.tile([128, 8, RCH, W], F16, name="dstack",
                                    tag="dstack")
                diff = [dstack[:, k] for k in range(8)]
                # 4 fused subtracts, 2 neighbors each (k-dim in the AP); the
                # center operand broadcasts over k with a 0-stride dim.
                # For the first chunk, emit in two row-halves so the first
                # half's diffs start as soon as the first conv chunks land.
                eflat = dpadE[:].rearrange("p r w -> p (r w)")
                oflat = dpadO[:].rearrange("p r w -> p (r w)")
                pairs = [  # (src flat view, base row offset, k step)
                    (eflat, rr0 - 1, 0, 2),        # (0,0),(0,2)
                    (eflat, rr0 + 1, 0, 2),        # (2,0),(2,2)
                    (eflat, rr0, 0, 2),            # (1,0),(1,2)
                    (oflat, rr0 - 1, 2, 2 * WP),   # (0,1),(2,1)
                ]
                halves = [(0, RCH // 2), (RCH // 2, RCH - RCH // 2)] \
                    if ci == 0 else [(0, RCH)]
                for hr0, hnr in halves:
                    ctrb = bass.AP(tensor=oflat.tensor,
                                   offset=(rr0 + hr0) * WP + 2,
                                   ap=[oflat.ap[0], [0, 2], [WP, hnr], [1, W]])
                    for pi, (src, brow, bcol, kst) in enumerate(pairs):
                        nb2 = bass.AP(tensor=src.tensor,
                                      offset=(brow + hr0) * WP + bcol,
                                      ap=[src.ap[0], [kst, 2], [WP, hnr],
                                          [1, W]])
                        v.tensor_tensor(dstack[:, 2 * pi:2 * pi + 2,
                                               hr0:hr0 + hnr], nb2, ctrb,
                                        ALU.subtract)


                absd = []
                for k in range(8):
                    ak = newt(f"absd{k}")
                    s.activation(ak[:], diff[k][:], AFT.Abs)   # ACT (DVE offload)
                    absd.append(ak)

                # dv + T1 (PE work, ACT evacs) emitted after the abs ops so
                # ACT reaches the abs stream before the network needs it;
                # T1 is only consumed by conv2 at the end of the chunk
                for ch in range(*DV_RANGES[ci]):
                    dv_chunk(ch)
                    t1_group(ch, T1sb, ch - 4 * ci)

                def tt(op, a, b, o):
                    v.tensor_tensor(o[:], a[:], b[:], op)
                    return o

                # 25-op selection of 4th-smallest-of-8 via median-of-9
                # (the 9th value is the always-zero center diff):
                # med9 = med3( max3(mins), med3(mids), min3(maxs) ) over three
                # sorted triples T0=(0,a0,a1), T1=(a2,a3,a4), T2=(a5,a6,a7).
                # absd slots are reused as scratch once their value is dead.
                A = absd
                p0, q0 = newt("p0"), newt("q0")
                tt(ALU.min, A[0], A[1], p0); tt(ALU.max, A[0], A[1], q0)
                # sort3 of (A2,A3,A4) -> lo1=A4, mi1=A0, hi1=A2
                tt(ALU.min, A[2], A[3], A[0]); tt(ALU.max, A[2], A[3], A[1])
                tt(ALU.max, A[1], A[4], A[2]); tt(ALU.min, A[1], A[4], A[3])
                tt(ALU.min, A[0], A[3], A[4]); tt(ALU.max, A[0], A[3], A[0])
                # sort3 of (A5,A6,A7) -> lo2=A7, mi2=A1, hi2=A5
                tt(ALU.min, A[5], A[6], A[1]); tt(ALU.max, A[5], A[6], A[3])
                tt(ALU.max, A[3], A[7], A[5]); tt(ALU.min, A[3], A[7], A[6])
                tt(ALU.min, A[1], A[6], A[7]); tt(ALU.max, A[1], A[6], A[1])
                # combine
                tt(ALU.max, A[4], A[7], A[3])          # mxlo
                tt(ALU.min, A[2], A[5], A[6])          # min(hi1,hi2)
                tt(ALU.min, A[6], q0, A[6])            # mnhi
                tt(ALU.min, p0, A[0], A[4])            # m_ab
                tt(ALU.max, p0, A[0], A[2])            # M_ab
                tt(ALU.min, A[2], A[1], A[7])          # m_c
                tt(ALU.max, A[4], A[7], A[4])          # mdmid
                tt(ALU.min, A[3], A[4], A[0])          # f_ab
                tt(ALU.max, A[3], A[4], A[1])          # F_ab
                tt(ALU.min, A[1], A[6], A[2])          # f_c
                # final comparator fused with the ETA floor, fp32 out; then
                # reciprocal in place -> medf holds 1/max(med, ETA).  All on
                # DVE: no ACT cast hops on the chunk critical path.
                medf = newt("medf", F32)
                v._custom_dve(MAXMAXETA, out=medf[:], in0=A[0][:], in1=A[2][:],
                              s0=ETA)
                v.reciprocal_approx_fast(medf[:], medf[:])

                # q = R/med per pixel via the fused prefix-scan op: stream
                # dstack with k innermost per pixel, accumulate
                # d*min(|d|*rmed, 1); q = difference of consecutive pixels'
                # k=7 prefix slots (row 0 of S stays zero for pixel 0).
                dsf = dstack[:].rearrange("p k r w -> p (k r w)")
                rmf = medf[:].rearrange("p r w -> p (r w)")
                last = ci == 64 // RCH - 1

                def scan_group(gi, row0, nrows):
                    off = W * row0
                    npx = W * nrows
                    din = bass.AP(tensor=dsf.tensor, offset=dsf.offset + off,
                                  ap=[dsf.ap[0], [1, npx], [RCH * W, 8]])
                    rin = bass.AP(tensor=rmf.tensor, offset=rmf.offset + off,
                                  ap=[rmf.ap[0], [1, npx], [0, 8]])
                    S = Stiles[gi % 2]
                    v._custom_dve(MEDSCAN, out=S[:, 1:npx + 1, :],
                                  in0=din, in1=rin)

                # ---- conv2: out = w2a^T d + w2b^T T1 - w2b^T (S_hi - S_lo)
                # + b2.  q = R/med = S_hi - S_lo and d3 = T1 - q both ride
                # the PSUM accumulation (w2bn = -w2b), so the DVE is done
                # with a group the moment its scan finishes.  Per-4-row
                # groups so the tail pipeline starts immediately; the last
                # chunk ends with 2-row groups to shorten the exit chain.
                osb = opool.tile([128, RCH, W], F32, name="osb", tag="osb")
                groups = [(0, 4), (4, 4), (8, 4), (12, 4)] if not last else \
                    [(0, 4), (4, 4), (8, 4), (12, 2), (14, 2)]
                for gi, (gr0, gnr) in enumerate(groups):
                    scan_group(gi, gr0, gnr)
                    S = Stiles[gi % 2]
                    Sf = S[:].rearrange("p n k -> p (n k)")
                    hi = bass.AP(tensor=Sf.tensor, offset=Sf.offset + 15,
                                 ap=[Sf.ap[0], [8 * W, gnr], [8, W]])
                    lo = bass.AP(tensor=Sf.tensor, offset=Sf.offset + 7,
                                 ap=[Sf.ap[0], [8 * W, gnr], [8, W]])
                    sl = slice(gr0, gr0 + gnr)
                    r0 = rr0 + gr0
                    ps2 = pp2.tile([128, gnr, W], F32, name="ps2", tag="ps2")
                    nc.tensor.matmul(ps2[:], w2asb[:],
                                     dpadE[:, r0:r0 + gnr, 1:129],
                                     start=True, stop=False)
                    nc.tensor.matmul(ps2[:], w2bsb[:], T1sb[:, sl],
                                     start=False, stop=False)
                    nc.tensor.matmul(ps2[:], w2bnf32[:], hi,
                                     start=False, stop=False)
                    nc.tensor.matmul(ps2[:], w2bf32[:], lo,
                                     start=False, stop=True)
                    s.add(osb[:, sl], ps2[:], b2sb[:])
                    if last:
                        g0 = ci * RCH + gr0
                        nc.sync.dma_start(out[:, g0:g0 + gnr, :], osb[0:64, sl])
                        nc.sync.dma_start(out[:, 64 + g0:64 + g0 + gnr, :],
                                          osb[64:128, sl])

                if not last:
                    g0 = ci * RCH
                    nc.sync.dma_start(out[:, g0:g0 + RCH, :], osb[0:64])
                    nc.sync.dma_start(out[:, 64 + g0:64 + g0 + RCH, :],
                                      osb[64:128])

    nc.compile()
    return nc


_NC_CACHE = None


def _get_program():
    global _NC_CACHE
    if _NC_CACHE is None:
        _NC_CACHE = build_program()
    return _NC_CACHE


def _host_inputs(x, w1, b1, w2, b2):
    """Build the per-core input maps (shard by batch, prep weights)."""
    f16 = np.float16
    w1t = np.ascontiguousarray(w1.T)                        # (c_in, c_out)
    w2at = np.ascontiguousarray(w2[:, :C].T)                # (c, o)
    w2bt = np.ascontiguousarray(w2[:, C:].T)
    bd = lambda m: np.block([[m, np.zeros_like(m)], [np.zeros_like(m), m]]).astype(f16)
    w1bd, w2abd, w2bbd = bd(w1t), bd(w2at), bd(w2bt)
    w2bnbd = bd(-w2bt)
    w1n9bd = bd(-9.0 * w1t)
    identbd = np.eye(128, dtype=f16)
    b1v = np.concatenate([b1, b1]).astype(np.float32).reshape(128, 1)
    b2v = np.concatenate([b2, b2]).astype(np.float32).reshape(128, 1)
    in_maps = []
    for i in range(NCORES):
        in_maps.append({
            "x16": np.ascontiguousarray(x[i].astype(f16)),
            "w1bd": w1bd, "w2abd": w2abd, "w2bbd": w2bbd,
            "w2bnbd": w2bnbd, "w1n9bd": w1n9bd, "identd": identbd,
            "w2bf": bd(w2bt).astype(np.float32),
            "w2bnf": bd(-w2bt).astype(np.float32),
            "b1v": b1v, "b2v": b2v,
        })
    return in_maps


def _spot_check(out, x, w1, b1, w2, b2, b=0, h=5):
    """Host-side reference for one output row; guards against the axon
    relay's rare whole-invocation garbage (seen once: rel ~37 vs 6e-3,
    clean on rerun).  Loose threshold: true fp16 rel_l2 is ~4e-3."""
    d = np.einsum('oc,chw->ohw', w1, x[b, :, h - 1:h + 2, :]) \
        + b1[:, None, None]
    p = np.pad(d, ((0, 0), (0, 0), (1, 1)), mode='reflect')
    nb = np.stack([p[:, i, j:j + 128] for i in range(3) for j in range(3)],
                  axis=-1)
    diff = nb - d[:, 1:2, :].transpose(0, 2, 1)
    absd = np.abs(diff)
    med = np.median(absd, axis=-1, keepdims=True)
    keep = absd <= med
    dz = np.where(keep, absd, 0.0)
    s = dz / np.maximum(dz.max(axis=-1, keepdims=True), 1e-30)
    d3 = (np.where(keep, diff, 0.0) * (1.0 - s)).sum(-1)
    cat = np.concatenate([d[:, 1, :], d3], axis=0)
    ref = np.einsum('oc,cw->ow', w2, cat) + b2[:, None]
    got = out[b, :, h, :]
    denom = np.linalg.norm(ref) + 1e-30
    return np.linalg.norm(got - ref) / denom


def kernel(x, w1, b1, w2, b2):
    x = np.asarray(x, dtype=np.float32)
    w1 = np.asarray(w1, dtype=np.float32)
    b1 = np.asarray(b1, dtype=np.float32)
    w2 = np.asarray(w2, dtype=np.float32)
    b2 = np.asarray(b2, dtype=np.float32)
    nc = _get_program()
    in_maps = _host_inputs(x, w1, b1, w2, b2)
    for attempt in range(3):
        res = run_bass_kernel_spmd(nc, in_maps, core_ids=list(range(NCORES)))
        out = np.stack([res.results[i]["out"] for i in range(NCORES)], axis=0)
        if _spot_check(out, x, w1, b1, w2, b2) < 0.05:
            break
    return out.astype(np.float32)

